# revision 1
# baseline (speedup 1.0000x reference)
"""Trainium2 Bass kernel for nn_ContrastLoss_Disentangle.

Contract: kernel(**inputs) takes the FULL (unsharded) inputs and returns the
same structure the reference returns: (loss_label, loss_norm, loss_triple)
as float32 scalars.

Pipeline (8 NeuronCores, data-parallel):
  host:    pose norms + normalization + [C*D, Np] transpose (poseFT)
  device1: per-core: nlp-row sumsq (ACT square+accum), raw nlp.pose dots
           (DVE mul + grouped reduce), and a [512, 1024] block of the
           pm gram matrix (PE, f32r fast path)
  host:    scores + BCE, pm assembly + stable argsort rank-select (furthest)
  device2: per-core dots of the gathered "hard positive" nlp rows
  host:    triplet loss assembly
"""

import os
import numpy as np

import concourse.bass as bass
import concourse.tile as tile
from concourse import bacc, mybir
from concourse.bass2jax import install_neuronx_cc_hook, partition_id_tensor, _bass_exec_p

C, NP, K, D = 8, 2048, 4, 256
NN = NP * K          # 8192
NCORES = 8
NPL = NP // NCORES   # 256 poses per core
NNL = NN // NCORES   # 1024 nlp rows per core
NT = NNL // 128      # 8 nlp tiles per category per core
CD = C * D           # 2048 contraction size

# pm block grid: 4 row-blocks x 2 col-blocks
PM_MI, PM_NJ = 4, 2
PM_M = NP // PM_MI   # 512 rows per core block
PM_N = NP // PM_NJ   # 1024 cols per core block

PM_MODE = os.environ.get("PM_MODE", "f32r")  # "f32r" | "bf16split" | "f32"

_runners = {}


def _build_dots_kernel(with_pm: bool, with_ssq: bool = True):
    """Per-core program. Inputs (per core):
      nlp   [C, NNL, D] f32   (raw nlp rows of this core; pose-major blocks)
      pose  [C, NPL, D] f32   (normalized pose rows matching this core's nlps)
      pm_l / pm_r             (poseFT column slices; only when with_pm)
    Outputs:
      ssq  [128, C*NT] f32    sumsq of nlp rows (col = (cat*2+pt)*4+k)
      rdot [128, C*NT] f32    dot(nlp_row, poseF[row//4]), same layout
      pmblk [PM_M, PM_N] f32  (only when with_pm)
    """
    nc = bacc.Bacc("TRN2", target_bir_lowering=False, debug=False,
                   num_devices=NCORES)
    nlp = nc.dram_tensor("nlp", [C, NNL, D], mybir.dt.float32,
                         kind="ExternalInput").ap()
    pose = nc.dram_tensor("pose", [C, NPL, D], mybir.dt.float32,
                          kind="ExternalInput").ap()
    if with_pm:
        if PM_MODE == "bf16split":
            pml_h = nc.dram_tensor("pml_h", [CD, PM_M], mybir.dt.bfloat16, kind="ExternalInput").ap()
            pml_l = nc.dram_tensor("pml_l", [CD, PM_M], mybir.dt.bfloat16, kind="ExternalInput").ap()
            pmr_h = nc.dram_tensor("pmr_h", [CD, PM_N], mybir.dt.bfloat16, kind="ExternalInput").ap()
            pmr_l = nc.dram_tensor("pmr_l", [CD, PM_N], mybir.dt.bfloat16, kind="ExternalInput").ap()
        else:
            pmdt = mybir.dt.float32r if PM_MODE == "f32r" else mybir.dt.float32
            pm_l = nc.dram_tensor("pm_l", [CD, PM_M], pmdt, kind="ExternalInput").ap()
            pm_r = nc.dram_tensor("pm_r", [CD, PM_N], pmdt, kind="ExternalInput").ap()
        pmblk = nc.dram_tensor("pmblk", [PM_M, PM_N], mybir.dt.float32,
                               kind="ExternalOutput").ap()
    if with_ssq:
        ssq = nc.dram_tensor("ssq", [128, C * NT], mybir.dt.float32,
                             kind="ExternalOutput").ap()
    rdot = nc.dram_tensor("rdot", [128, C * NT], mybir.dt.float32,
                          kind="ExternalOutput").ap()

    with tile.TileContext(nc) as tc:
        with tc.tile_pool(name="io", bufs=int(os.environ.get("BUFS_IO", 6))) as io, \
             tc.tile_pool(name="pose_p", bufs=3) as pose_p, \
             tc.tile_pool(name="scr", bufs=4) as scr, \
             tc.tile_pool(name="accum", bufs=1) as accum, \
             tc.tile_pool(name="matres", bufs=1) as matres, \
             tc.tile_pool(name="rhs_p", bufs=int(os.environ.get("BUFS_RHS", 6))) as rhs_p, \
             tc.tile_pool(name="ev", bufs=3) as ev, \
             tc.tile_pool(name="ps", bufs=1, space="PSUM") as ps:

            if with_ssq:
                ssq_t = accum.tile([128, C * NT], mybir.dt.float32, tag="ssq")
                nc.gpsimd.memset(ssq_t[:], 0.0)
            rdot_t = accum.tile([128, C * NT], mybir.dt.float32, tag="rdot")
            nc.gpsimd.memset(rdot_t[:], 0.0)

            # ---------- sub-programs -------------------------------------
            def a_iter(i):
                cat, pt = i // 2, i % 2
                col4 = (cat * (NPL // 128) + pt) * K
                po = pose_p.tile([128, D], mybir.dt.float32, tag="po",
                                 name=f"po{i}")
                nc.sync.dma_start(po[:], pose[cat, 128 * pt:128 * (pt + 1), :])
                x = io.tile([128, K * D], mybir.dt.float32, tag="x",
                            name=f"x{i}")
                nc.sync.dma_start(
                    x[:], nlp[cat, 512 * pt:512 * (pt + 1), :]
                    .rearrange("(p k) d -> p k d", k=K))
                full = po[:]
                rep = bass.AP(tensor=full.tensor, offset=full.offset,
                              ap=[list(full.ap[0]), [0, K], [1, D]])
                z = scr.tile([128, K * D], mybir.dt.float32, tag="z",
                             name=f"z{i}")
                nc.vector.tensor_tensor(
                    z[:].rearrange("p (k d) -> p k d", k=K),
                    x[:].rearrange("p (k d) -> p k d", k=K),
                    rep, op=mybir.AluOpType.mult)
                if with_ssq:
                    # dots reduce on DVE, sumsq on ACT
                    nc.vector.tensor_reduce(
                        rdot_t[:, col4:col4 + K],
                        z[:].rearrange("p (k d) -> p k d", k=K),
                        axis=mybir.AxisListType.X, op=mybir.AluOpType.add)
                    s1 = scr.tile([128, D], mybir.dt.float32, tag="s1",
                                  name=f"s1_{i}")
                    for k in range(K):
                        nc.scalar.activation(
                            s1[:], x[:, D * k:D * (k + 1)],
                            mybir.ActivationFunctionType.Square,
                            accum_out=ssq_t[:, col4 + k:col4 + k + 1])
                elif i % 2 == 0:
                    # no sumsq needed -> alternate the dots reduce between
                    # the idle ACT (copy+accum) and the DVE (grouped reduce)
                    s1 = scr.tile([128, D], mybir.dt.float32, tag="s1",
                                  name=f"s1_{i}")
                    for k in range(K):
                        nc.scalar.activation(
                            s1[:], z[:, D * k:D * (k + 1)],
                            mybir.ActivationFunctionType.Copy,
                            accum_out=rdot_t[:, col4 + k:col4 + k + 1])
                else:
                    nc.vector.tensor_reduce(
                        rdot_t[:, col4:col4 + K],
                        z[:].rearrange("p (k d) -> p k d", k=K),
                        axis=mybir.AxisListType.X, op=mybir.AluOpType.add)

            accs = {}

            def pm_chunk(s):
                n, k = s // 16, s % 16
                KT = CD // 128
                if k == 0:
                    accs[n] = [ps.tile([128, 512], mybir.dt.float32,
                                       name=f"acc{n}{m}", tag=f"acc{n}{m}")
                               for m in range(PM_MI)]
                rt = rhs_p.tile([128, 512], pm_rdt, tag="rt", name=f"rt{s}")
                nc.sync.dma_start(rt[:], pm_r[128 * k:128 * (k + 1),
                                              512 * n:512 * (n + 1)])
                for m in range(PM_MI):
                    nc.tensor.matmul(accs[n][m][:],
                                     lt[:, k, 128 * m:128 * (m + 1)], rt[:],
                                     start=(k == 0), stop=(k == KT - 1))
                if k == KT - 1:
                    for m in range(PM_MI):
                        o = ev.tile([128, 512], mybir.dt.float32, tag="ev",
                                    name=f"ev{n}{m}")
                        nc.scalar.copy(o[:], accs[n][m][:])
                        nc.sync.dma_start(
                            pmblk[128 * m:128 * (m + 1),
                                  512 * n:512 * (n + 1)], o[:])

            # ---------- emission order: interleave pm with dots ----------
            if with_pm:
                assert PM_MODE in ("f32r", "f32"), "bf16split path removed"
                pm_rdt = mybir.dt.float32r if PM_MODE == "f32r" else mybir.dt.float32
                KT = CD // 128
                _order = os.environ.get("K1_ORDER", "A")
                # unblock DVE/ACT before the big lhsT load hits the queues
                a_iter(0)
                a_iter(1)
                lt = matres.tile([128, KT, PM_M], pm_rdt, tag="lt")
                # split the big lhsT load into 4 chunks
                for kc in range(4):
                    nc.sync.dma_start(
                        lt[:, 4 * kc:4 * (kc + 1), :],
                        pm_l[512 * kc:512 * (kc + 1), :]
                        .rearrange("(k p) m -> p k m", p=128))
                for s in range(32):
                    pm_chunk(s)
                    if _order == "A":
                        if s % 2 == 0 and 2 + s // 2 < 16:
                            a_iter(2 + s // 2)
                    else:
                        if 2 + s < 16:
                            a_iter(2 + s)
            else:
                for i in range(16):
                    a_iter(i)

            if with_ssq:
                nc.sync.dma_start(ssq[:], ssq_t[:])
            nc.sync.dma_start(rdot[:], rdot_t[:])

    nc.finalize()
    return nc


def _make_runner(nc):
    """Reusable jitted SPMD runner (replicates bass2jax.run_bass_via_pjrt but
    caches the compiled executable across calls)."""
    import jax
    from jax.sharding import Mesh, PartitionSpec
    from jax.experimental.shard_map import shard_map

    install_neuronx_cc_hook()
    partition_name = nc.partition_id_tensor.name if nc.partition_id_tensor else None
    in_names, out_names, out_avals = [], [], []
    for alloc in nc.m.functions[0].allocations:
        if not isinstance(alloc, mybir.MemoryLocationSet):
            continue
        name = alloc.memorylocations[0].name
        if alloc.kind == "ExternalInput":
            if name != partition_name:
                in_names.append(name)
        elif alloc.kind == "ExternalOutput":
            out_names.append(name)
            out_avals.append(jax.core.ShapedArray(
                tuple(alloc.tensor_shape), mybir.dt.np(alloc.dtype)))
    n_params = len(in_names)
    all_in = in_names + out_names + ([partition_name] if partition_name else [])

    def _body(*args):
        operands = list(args)
        if partition_name is not None:
            operands.append(partition_id_tensor())
        outs = _bass_exec_p.bind(
            *operands, out_avals=tuple(out_avals), in_names=tuple(all_in),
            out_names=tuple(out_names), lowering_input_output_aliases=(),
            sim_require_finite=False, sim_require_nnan=False, nc=nc)
        return tuple(outs)

    devices = jax.devices()[:NCORES]
    mesh = Mesh(np.asarray(devices), ("core",))
    donate = tuple(range(n_params, n_params + len(out_names)))
    sharded = jax.jit(
        shard_map(_body, mesh=mesh,
                  in_specs=(PartitionSpec("core"),) * (n_params + len(out_names)),
                  out_specs=(PartitionSpec("core"),) * len(out_names),
                  check_rep=False),
        donate_argnums=donate, keep_unused=True)

    def run(in_maps):
        concat_in = [np.concatenate([np.asarray(m[name]) for m in in_maps], axis=0)
                     for name in in_names]
        zeros = [np.zeros((NCORES * a.shape[0], *a.shape[1:]), a.dtype)
                 for a in out_avals]
        out_arrs = sharded(*concat_in, *zeros)
        return [
            {name: np.asarray(out_arrs[i]).reshape(NCORES, *out_avals[i].shape)[c]
             for i, name in enumerate(out_names)}
            for c in range(NCORES)
        ]

    return run


def _get_runner(key):
    if key not in _runners:
        if key == "k1":
            _runners[key] = _make_runner(_build_dots_kernel(with_pm=True))
        else:
            _runners[key] = _make_runner(
                _build_dots_kernel(with_pm=False, with_ssq=False))
    return _runners[key]


def _col_to_rows(a):
    """[8 cores][128, C*NT] device output -> [C, NN] (global nlp rows).

    column = (cat*2 + pt)*K + k ; partition p -> nlp row 512*pt + 4*p + k
    within the core shard (pose-major layout)."""
    out = np.empty((C, NN), np.float32)
    for c in range(NCORES):
        blk = a[c].reshape(128, C, NPL // 128, K)      # [p, cat, pt, k]
        out[:, c * NNL:(c + 1) * NNL] = (
            blk.transpose(1, 2, 0, 3).reshape(C, NNL))
    return out


def _kernel_host_fallback(inputs):
    """Pure-numpy reference replication, used only if the index tensors do
    not have the canonical arange structure the device layout relies on."""
    nlp = np.asarray(inputs["nlp_features"], np.float32)
    pose = np.asarray(inputs["pose_features"], np.float32)
    nlab = np.asarray(inputs["nlp_label"]).astype(np.int64)
    n2p = np.asarray(inputs["nlpid2poseid"]).astype(np.int64)
    p2n = np.asarray(inputs["pose2nlpid"]).astype(np.int64)
    cat = np.asarray(inputs["categories"], np.float32)
    ri = np.asarray(inputs["rand_index"]).astype(np.int64)
    Np, Nn = pose.shape[1], nlp.shape[1]
    norm_p = np.sqrt(np.einsum("cpd,cpd->cp", pose, pose, dtype=np.float32))
    norm_n = np.sqrt(np.einsum("cnd,cnd->cn", nlp, nlp, dtype=np.float32))
    poseF = pose / norm_p[:, :, None]
    nlpF = nlp / norm_n[:, :, None]
    loss_norm = np.float32(np.float32(norm_p.mean()) + np.float32(norm_n.mean()))
    dots = np.einsum("cnd,cnd->cn", nlpF, poseF[:, n2p]).astype(np.float32)
    scores = np.einsum("cn,nc->n", dots, cat).astype(np.float32)
    p = (1.0 / (1.0 + np.exp(-scores))).astype(np.float32)
    lblf = nlab.astype(np.float32)
    loss_label = np.float32(
        np.mean(-(np.log(p) * lblf + np.log(1.0 - p) * (1.0 - lblf))))
    pf = np.ascontiguousarray(poseF.transpose(0, 2, 1).reshape(-1, Np))
    pm = (pf.T @ pf).astype(np.float32)
    ar = np.arange(Np)
    pm[ar, ar] = 1.0
    order = np.argsort(pm, axis=1, kind="stable")
    furthest = order[ar, ri]
    sg = scores[p2n]
    lg = nlab[p2n]
    maxp = np.maximum(np.max(np.where(lg == 0, sg, -np.inf), axis=1), -1.0)
    minp = np.minimum(np.min(np.where(lg == 1, sg, np.inf), axis=1), 1.0)
    nids = p2n[furthest]
    cd = np.einsum("cpkd,cpd->cpk", nlpF[:, nids], poseF)
    cur = np.einsum("cpk,pkc->pk", cd, cat[nids]).astype(np.float32)
    lcur = nlab[nids]
    maxcur = np.max(np.where(lcur == 1, cur, -np.inf), axis=1)
    maxp = np.maximum(maxp, maxcur)
    found = ~((maxp == -1.0) | (minp == 1.0))
    lt = np.where(found, maxp - minp + 2.0, 0.0).astype(np.float32)
    nf = int(np.sum(~found))
    loss_triple = (np.float32(0.0) if nf == Nn else
                   np.float32(lt.sum(dtype=np.float32) / np.float32(Nn - nf)))
    return (np.float32(loss_label), loss_norm, loss_triple)


def kernel(**inputs):
    nlp = np.ascontiguousarray(inputs["nlp_features"], np.float32)      # [C, NN, D]
    pose = np.ascontiguousarray(inputs["pose_features"], np.float32)    # [C, NP, D]
    nlab = np.asarray(inputs["nlp_label"]).astype(np.int64)
    cat = np.ascontiguousarray(inputs["categories"], np.float32)        # [NN, C]
    ri = np.asarray(inputs["rand_index"]).astype(np.int64)

    n2p = np.asarray(inputs["nlpid2poseid"]).astype(np.int64)
    p2n = np.asarray(inputs["pose2nlpid"]).astype(np.int64)
    if (not np.array_equal(n2p, np.arange(NN) // K)
            or not np.array_equal(p2n, np.arange(NN).reshape(NP, K))):
        return _kernel_host_fallback(inputs)

    # ---- host: pose normalization + poseFT ------------------------------
    norm_p = np.sqrt(np.einsum("cpd,cpd->cp", pose, pose, dtype=np.float32,
                               optimize=True)).astype(np.float32)       # [C, NP]
    poseF = pose / norm_p[:, :, None]
    poseFT = np.ascontiguousarray(
        poseF.transpose(0, 2, 1).reshape(CD, NP))                       # [CD, NP]

    # ---- device kernel 1 -------------------------------------------------
    run1 = _get_runner("k1")
    in_maps = []
    for c in range(NCORES):
        i, j = c // PM_NJ, c % PM_NJ
        m = {
            "nlp": nlp[:, c * NNL:(c + 1) * NNL, :],
            "pose": poseF[:, c * NPL:(c + 1) * NPL, :],
            "pm_l": np.ascontiguousarray(poseFT[:, i * PM_M:(i + 1) * PM_M]),
            "pm_r": np.ascontiguousarray(poseFT[:, j * PM_N:(j + 1) * PM_N]),
        }
        in_maps.append(m)
    res1 = run1(in_maps)

    ssq_n = _col_to_rows([r["ssq"] for r in res1])                      # [C, NN]
    rdot = _col_to_rows([r["rdot"] for r in res1])                      # [C, NN]
    pm = np.empty((NP, NP), np.float32)
    for c in range(NCORES):
        i, j = c // PM_NJ, c % PM_NJ
        pm[i * PM_M:(i + 1) * PM_M, j * PM_N:(j + 1) * PM_N] = res1[c]["pmblk"]

    # ---- host: norms / scores / BCE -------------------------------------
    norm_n = np.sqrt(ssq_n)                                             # [C, NN]
    loss_norm = np.float32(np.float32(norm_p.mean()) + np.float32(norm_n.mean()))

    dots = (rdot / norm_n).astype(np.float32)                           # [C, NN]
    scores = np.einsum("cn,nc->n", dots, cat).astype(np.float32)
    p = (1.0 / (1.0 + np.exp(-scores))).astype(np.float32)
    lblf = nlab.astype(np.float32)
    loss_label = np.float32(
        np.mean(-(np.log(p) * lblf + np.log(1.0 - p) * (1.0 - lblf))))

    # ---- host: furthest selection ---------------------------------------
    ar = np.arange(NP)
    pm[ar, ar] = 1.0
    order = np.argsort(pm, axis=1, kind="stable")
    furthest = order[ar, ri]                                            # [NP]

    sg = scores.reshape(NP, K)
    lg = nlab.reshape(NP, K)
    maxp = np.maximum(np.max(np.where(lg == 0, sg, -np.inf), axis=1), -1.0)
    minp = np.minimum(np.min(np.where(lg == 1, sg, np.inf), axis=1), 1.0)

    nids = (furthest[:, None] * K + np.arange(K)).reshape(-1)           # [NN]

    # ---- device kernel 2: dots of gathered hard-positive rows ------------
    run2 = _get_runner("k2")
    gnlp = nlp[:, nids, :]                                              # [C, NN, D]
    in_maps2 = []
    for c in range(NCORES):
        in_maps2.append({
            "nlp": gnlp[:, c * NNL:(c + 1) * NNL, :],
            "pose": poseF[:, c * NPL:(c + 1) * NPL, :],
        })
    res2 = run2(in_maps2)
    crdot = _col_to_rows([r["rdot"] for r in res2])                     # [C, NN]

    cur_dots = (crdot / norm_n[:, nids]).astype(np.float32)             # [C, NN]
    cur = np.einsum("cn,nc->n", cur_dots, cat[nids]).astype(np.float32)
    cur = cur.reshape(NP, K)
    lcur = nlab[nids].reshape(NP, K)
    maxcur = np.max(np.where(lcur == 1, cur, -np.inf), axis=1)
    maxp = np.maximum(maxp, maxcur)
    found = ~((maxp == -1.0) | (minp == 1.0))
    lt = np.where(found, maxp - minp + 2.0, 0.0).astype(np.float32)
    not_find = int(np.sum(~found))
    if not_find == NN:
        loss_triple = np.float32(0.0)
    else:
        loss_triple = np.float32(lt.sum(dtype=np.float32) / np.float32(NN - not_find))

    return (np.float32(loss_label), np.float32(loss_norm), np.float32(loss_triple))



# revision 2
# speedup vs baseline: 2.7178x; 2.7178x over previous
"""Trainium2 Bass kernel for nn_ContrastLoss_Disentangle.

Contract: kernel(**inputs) takes the FULL (unsharded) inputs and returns the
same structure the reference returns: (loss_label, loss_norm, loss_triple)
as float32 scalars.

Pipeline (8 NeuronCores, data-parallel):
  host:    norms (exact), normalization, categories folded into nlp rows
           (g = nlpF * cat), everything scaled x16 and cast to fp8_e4m3,
           transposed to [C*D, rows] layouts pre-swizzled for SBUF
  device1: per-core: scores via fp8 DoubleRow PE matmuls (diag-extracted
           with a mask on DVE) + a [512, 1024] block of the pm gram matrix
           (fp8 DoubleRow PE, fp16 output)
  host:    BCE, stable argsort rank-select (furthest), gather of the
           "hard positive" g columns
  device2: per-core: diag-extracted dots of the gathered columns
  host:    triplet loss assembly

All heavy HBM traffic is fp8 (nlp rows: 2 MB/core, pose gram operands:
3 MB/core) and the pm block leaves as fp16, which puts the kernel near the
DMA roofline of the part.  Accuracy: fp8 x16 rounding perturbs scores by
~0.5% absolute and reshuffles `furthest` between near-rank poses; measured
end-to-end max rel err ~3e-5 (gate 2e-2).
"""

import os
import numpy as np
import ml_dtypes

import concourse.bass as bass
import concourse.tile as tile
from concourse import bacc, mybir
from concourse.bass2jax import install_neuronx_cc_hook, partition_id_tensor, _bass_exec_p

C, NP, K, D = 8, 2048, 4, 256
NN = NP * K          # 8192
NCORES = 8
NPL = NP // NCORES   # 256 poses per core
NNL = NN // NCORES   # 1024 nlp rows per core
CD = C * D           # 2048 contraction size
KT = CD // 128       # 16 k-tiles

SC = float(os.environ.get("FP8_SCALE", "16"))
F8 = ml_dtypes.float8_e4m3
DR = os.environ.get("DR", "1") == "1"

# pm block grid: 4 row-blocks x 2 col-blocks
PM_MI, PM_NJ = 4, 2
PM_M = NP // PM_MI   # 512 rows per core block
PM_N = NP // PM_NJ   # 1024 cols per core block

_runners = {}


def _build_kernel(with_pm: bool):
    """Per-core program. Inputs (per core, all pre-swizzled to [128, KT, W]
    with partition p holding contraction dims k*128+p):
      g    [128, KT, 1024] fp8   nlp-side columns; col j = 512*hh+128*m+p
                                 maps to local nlp row 512*hh+4*p+m
      hl   [128, KT, 512|256] fp8  pose columns; first 256 are this core's
                                 own poses (k1 rows permuted, see host)
      hr   [128, KT, 1024] fp8   pm rhs pose columns (only when with_pm)
      mask [128, 512] bf16       mask[p, 128*m+q] = (q == p)
    Outputs:
      sc   [128, 8] f32          raw dots, col = hh*4 + m
      pm16 [512, 1024] fp16      pm block (only when with_pm)
    """
    nc = bacc.Bacc("TRN2", target_bir_lowering=False, debug=False,
                   num_devices=NCORES)
    f8 = mybir.dt.float8e4
    hlw = PM_M if with_pm else NPL
    g_in = nc.dram_tensor("g", [128, KT, NNL], f8, kind="ExternalInput").ap()
    hl_in = nc.dram_tensor("hl", [128, KT, hlw], f8, kind="ExternalInput").ap()
    if with_pm:
        hr_in = nc.dram_tensor("hr", [128, KT, PM_N], f8,
                               kind="ExternalInput").ap()
        pmo = nc.dram_tensor("pm16", [PM_M, PM_N], mybir.dt.float16,
                             kind="ExternalOutput").ap()
    mask_in = nc.dram_tensor("mask", [128, 512], mybir.dt.bfloat16,
                             kind="ExternalInput").ap()
    sc_out = nc.dram_tensor("sc", [128, 8], mybir.dt.float32,
                            kind="ExternalOutput").ap()

    with tile.TileContext(nc) as tc:
        with tc.tile_pool(name="big", bufs=1) as big, \
             tc.tile_pool(name="scr", bufs=2) as scr, \
             tc.tile_pool(name="ev", bufs=3) as ev, \
             tc.tile_pool(name="ps", bufs=3, space="PSUM") as ps:

            mask_t = big.tile([128, 512], mybir.dt.bfloat16, tag="mask")
            nc.sync.dma_start(mask_t[:], mask_in)
            hl_t = big.tile([128, KT, hlw], f8, tag="hl")
            nc.sync.dma_start(hl_t[:], hl_in)
            g_t = big.tile([128, KT, NNL], f8, tag="g")
            nc.sync.dma_start(g_t[:, :, 0:512], g_in[:, :, 0:512])
            nc.sync.dma_start(g_t[:, :, 512:1024], g_in[:, :, 512:1024])
            if with_pm:
                hr_t = big.tile([128, KT, PM_N], f8, tag="hr")
                nc.sync.dma_start(hr_t[:, :, 0:512], hr_in[:, :, 0:512])
                nc.sync.dma_start(hr_t[:, :, 512:1024], hr_in[:, :, 512:1024])
            sc_t = big.tile([128, 8], mybir.dt.float32, tag="sc")

            def mm(acc, lcols, rt, rcols):
                if DR:
                    for kp in range(KT // 2):
                        nc.tensor.matmul(
                            acc[:],
                            hl_t[:, 2 * kp:2 * kp + 2, lcols:lcols + 128],
                            rt[:, 2 * kp:2 * kp + 2, rcols:rcols + 512],
                            start=(kp == 0), stop=(kp == KT // 2 - 1),
                            perf_mode=mybir.MatmulPerfMode.DoubleRow)
                else:
                    for k in range(KT):
                        nc.tensor.matmul(
                            acc[:],
                            hl_t[:, k, lcols:lcols + 128],
                            rt[:, k, rcols:rcols + 512],
                            start=(k == 0), stop=(k == KT - 1))

            # scores: diag blocks of (own poses)^T @ g
            for hh in range(2):
                acc_s = ps.tile([128, 512], mybir.dt.float32, tag="ps",
                                name=f"accs{hh}")
                mm(acc_s, 128 * hh, g_t, 512 * hh)
                z = scr.tile([128, 512], mybir.dt.float32, tag="z",
                             name=f"z{hh}")
                nc.vector.tensor_tensor(z[:], acc_s[:], mask_t[:],
                                        op=mybir.AluOpType.mult)
                nc.vector.tensor_reduce(
                    sc_t[:, 4 * hh:4 * hh + 4],
                    z[:].rearrange("p (m q) -> p m q", m=4),
                    axis=mybir.AxisListType.X, op=mybir.AluOpType.add)

            # pm block: hl^T @ hr, streamed per (nb, mb) through rotating psum
            if with_pm:
                for nb in range(2):
                    for mb in range(4):
                        acc = ps.tile([128, 512], mybir.dt.float32, tag="ps",
                                      name=f"acc{nb}{mb}")
                        mm(acc, 128 * mb, hr_t, 512 * nb)
                        o = ev.tile([128, 512], mybir.dt.float16, tag="ev",
                                    name=f"ev{nb}{mb}")
                        nc.scalar.copy(o[:], acc[:])
                        nc.sync.dma_start(
                            pmo[128 * mb:128 * (mb + 1),
                                512 * nb:512 * (nb + 1)], o[:])

            nc.sync.dma_start(sc_out, sc_t[:])

    nc.finalize()
    return nc


def _make_runner(nc):
    """Reusable jitted SPMD runner (replicates bass2jax.run_bass_via_pjrt but
    caches the compiled executable across calls)."""
    import jax
    from jax.sharding import Mesh, PartitionSpec
    from jax.experimental.shard_map import shard_map

    install_neuronx_cc_hook()
    partition_name = nc.partition_id_tensor.name if nc.partition_id_tensor else None
    in_names, out_names, out_avals = [], [], []
    for alloc in nc.m.functions[0].allocations:
        if not isinstance(alloc, mybir.MemoryLocationSet):
            continue
        name = alloc.memorylocations[0].name
        if alloc.kind == "ExternalInput":
            if name != partition_name:
                in_names.append(name)
        elif alloc.kind == "ExternalOutput":
            out_names.append(name)
            out_avals.append(jax.core.ShapedArray(
                tuple(alloc.tensor_shape), mybir.dt.np(alloc.dtype)))
    n_params = len(in_names)
    all_in = in_names + out_names + ([partition_name] if partition_name else [])

    def _body(*args):
        operands = list(args)
        if partition_name is not None:
            operands.append(partition_id_tensor())
        outs = _bass_exec_p.bind(
            *operands, out_avals=tuple(out_avals), in_names=tuple(all_in),
            out_names=tuple(out_names), lowering_input_output_aliases=(),
            sim_require_finite=False, sim_require_nnan=False, nc=nc)
        return tuple(outs)

    devices = jax.devices()[:NCORES]
    mesh = Mesh(np.asarray(devices), ("core",))
    donate = tuple(range(n_params, n_params + len(out_names)))
    sharded = jax.jit(
        shard_map(_body, mesh=mesh,
                  in_specs=(PartitionSpec("core"),) * (n_params + len(out_names)),
                  out_specs=(PartitionSpec("core"),) * len(out_names),
                  check_rep=False),
        donate_argnums=donate, keep_unused=True)

    def run(in_maps):
        concat_in = [np.concatenate([np.asarray(m[name]) for m in in_maps], axis=0)
                     for name in in_names]
        zeros = [np.zeros((NCORES * a.shape[0], *a.shape[1:]), a.dtype)
                 for a in out_avals]
        out_arrs = sharded(*concat_in, *zeros)
        return [
            {name: np.asarray(out_arrs[i]).reshape(NCORES, *out_avals[i].shape)[c]
             for i, name in enumerate(out_names)}
            for c in range(NCORES)
        ]

    return run


def _get_runner(key):
    if key not in _runners:
        _runners[key] = _make_runner(_build_kernel(with_pm=(key == "k1")))
    return _runners[key]


def _swz(x):
    """[CD, W] (uint8/fp8) -> [128, KT, W] with partition p holding
    contraction rows k*128+p."""
    return np.ascontiguousarray(x.reshape(KT, 128, x.shape[1]).transpose(1, 0, 2))


def _kernel_host_fallback(inputs):
    """Pure-numpy reference replication, used only if the index tensors do
    not have the canonical arange structure the device layout relies on."""
    nlp = np.asarray(inputs["nlp_features"], np.float32)
    pose = np.asarray(inputs["pose_features"], np.float32)
    nlab = np.asarray(inputs["nlp_label"]).astype(np.int64)
    n2p = np.asarray(inputs["nlpid2poseid"]).astype(np.int64)
    p2n = np.asarray(inputs["pose2nlpid"]).astype(np.int64)
    cat = np.asarray(inputs["categories"], np.float32)
    ri = np.asarray(inputs["rand_index"]).astype(np.int64)
    Np, Nn = pose.shape[1], nlp.shape[1]
    norm_p = np.sqrt(np.einsum("cpd,cpd->cp", pose, pose, dtype=np.float32))
    norm_n = np.sqrt(np.einsum("cnd,cnd->cn", nlp, nlp, dtype=np.float32))
    poseF = pose / norm_p[:, :, None]
    nlpF = nlp / norm_n[:, :, None]
    loss_norm = np.float32(np.float32(norm_p.mean()) + np.float32(norm_n.mean()))
    dots = np.einsum("cnd,cnd->cn", nlpF, poseF[:, n2p]).astype(np.float32)
    scores = np.einsum("cn,nc->n", dots, cat).astype(np.float32)
    p = (1.0 / (1.0 + np.exp(-scores))).astype(np.float32)
    lblf = nlab.astype(np.float32)
    loss_label = np.float32(
        np.mean(-(np.log(p) * lblf + np.log(1.0 - p) * (1.0 - lblf))))
    pf = np.ascontiguousarray(poseF.transpose(0, 2, 1).reshape(-1, Np))
    pm = (pf.T @ pf).astype(np.float32)
    ar = np.arange(Np)
    pm[ar, ar] = 1.0
    order = np.argsort(pm, axis=1, kind="stable")
    furthest = order[ar, ri]
    sg = scores[p2n]
    lg = nlab[p2n]
    maxp = np.maximum(np.max(np.where(lg == 0, sg, -np.inf), axis=1), -1.0)
    minp = np.minimum(np.min(np.where(lg == 1, sg, np.inf), axis=1), 1.0)
    nids = p2n[furthest]
    cd = np.einsum("cpkd,cpd->cpk", nlpF[:, nids], poseF)
    cur = np.einsum("cpk,pkc->pk", cd, cat[nids]).astype(np.float32)
    lcur = nlab[nids]
    maxcur = np.max(np.where(lcur == 1, cur, -np.inf), axis=1)
    maxp = np.maximum(maxp, maxcur)
    found = ~((maxp == -1.0) | (minp == 1.0))
    lt = np.where(found, maxp - minp + 2.0, 0.0).astype(np.float32)
    nf = int(np.sum(~found))
    loss_triple = (np.float32(0.0) if nf == Nn else
                   np.float32(lt.sum(dtype=np.float32) / np.float32(Nn - nf)))
    return (np.float32(loss_label), loss_norm, loss_triple)


def kernel(**inputs):
    nlp = np.ascontiguousarray(inputs["nlp_features"], np.float32)      # [C, NN, D]
    pose = np.ascontiguousarray(inputs["pose_features"], np.float32)    # [C, NP, D]
    nlab = np.asarray(inputs["nlp_label"]).astype(np.int64)
    cat = np.ascontiguousarray(inputs["categories"], np.float32)        # [NN, C]
    ri = np.asarray(inputs["rand_index"]).astype(np.int64)

    n2p = np.asarray(inputs["nlpid2poseid"]).astype(np.int64)
    p2n = np.asarray(inputs["pose2nlpid"]).astype(np.int64)
    if (not np.array_equal(n2p, np.arange(NN) // K)
            or not np.array_equal(p2n, np.arange(NN).reshape(NP, K))):
        return _kernel_host_fallback(inputs)

    # ---- host: exact norms, normalize, fold categories, fp8 x16 ---------
    norm_p = np.sqrt(np.einsum("cpd,cpd->cp", pose, pose, dtype=np.float32,
                               optimize=True)).astype(np.float32)       # [C, NP]
    norm_n = np.sqrt(np.einsum("cnd,cnd->cn", nlp, nlp, dtype=np.float32,
                               optimize=True)).astype(np.float32)       # [C, NN]
    loss_norm = np.float32(np.float32(norm_p.mean()) + np.float32(norm_n.mean()))

    poseF = pose / norm_p[:, :, None]
    hT8 = np.ascontiguousarray(
        (poseF * SC).transpose(0, 2, 1)).reshape(CD, NP).astype(F8)     # [CD, NP]

    gscale = (cat.T / norm_n) * SC                                      # [C, NN]
    g8 = (nlp * gscale[:, :, None]).astype(F8)                          # [C, NN, D]
    g8T = np.ascontiguousarray(g8.transpose(0, 2, 1)).reshape(CD, NN)   # [CD, NN]
    # (p, m) -> (m, p) within each 512-column block so the device's diag
    # mask lines up: col 512*hh + 128*m + p <- local row 512*hh + 4*p + m
    g8km = np.ascontiguousarray(
        g8T.reshape(CD, NN // 512, 128, 4).transpose(0, 1, 3, 2)
    ).reshape(CD, NN)

    mask = np.zeros((128, 512), np.float32)
    mask[np.arange(128)[:, None], 128 * np.arange(4)[None, :] + np.arange(128)[:, None]] = 1.0
    mask = mask.astype(ml_dtypes.bfloat16)

    # ---- device kernel 1 -------------------------------------------------
    # hl column order per core: own 256 poses first (so the scores matmul
    # can address them at a fixed offset), then the other half of the pm
    # row-block; pm rows are written back through rows_order.
    run1 = _get_runner("k1")
    in1 = []
    rows_order = np.empty((NCORES, PM_M), np.int64)
    for c in range(NCORES):
        i, j = c // PM_NJ, c % PM_NJ
        par = c % 2
        own = np.arange(512 * i + 256 * par, 512 * i + 256 * par + 256)
        oth = np.arange(512 * i + 256 * (1 - par), 512 * i + 256 * (1 - par) + 256)
        cols = np.concatenate([own, oth])
        rows_order[c] = cols
        in1.append({
            "g": _swz(g8km[:, c * NNL:(c + 1) * NNL]),
            "hl": _swz(hT8[:, cols]),
            "hr": _swz(hT8[:, PM_N * j:PM_N * (j + 1)]),
            "mask": mask,
        })
    res1 = run1(in1)

    # ---- host: scores / BCE ---------------------------------------------
    sc_all = np.stack([r["sc"] for r in res1])                          # [8,128,8]
    scores = (sc_all.reshape(NCORES, 128, 2, 4).transpose(0, 2, 1, 3)
              .reshape(NN) / np.float32(SC * SC)).astype(np.float32)
    p = (1.0 / (1.0 + np.exp(-scores))).astype(np.float32)
    lblf = nlab.astype(np.float32)
    loss_label = np.float32(
        np.mean(-(np.log(p) * lblf + np.log(1.0 - p) * (1.0 - lblf))))

    # ---- host: furthest selection ---------------------------------------
    pm = np.empty((NP, NP), np.float32)
    for c in range(NCORES):
        j = c % PM_NJ
        pm[rows_order[c], PM_N * j:PM_N * (j + 1)] = \
            res1[c]["pm16"].astype(np.float32)
    ar = np.arange(NP)
    pm[ar, ar] = np.float32(SC * SC)
    order = np.argsort(pm, axis=1, kind="stable")
    furthest = order[ar, ri]                                            # [NP]

    sg = scores.reshape(NP, K)
    lg = nlab.reshape(NP, K)
    maxp = np.maximum(np.max(np.where(lg == 0, sg, -np.inf), axis=1), -1.0)
    minp = np.minimum(np.min(np.where(lg == 1, sg, np.inf), axis=1), 1.0)

    # ---- device kernel 2: dots of gathered hard-positive columns ---------
    fq = furthest.reshape(NCORES, 2, 128)
    idx = (fq[:, :, None, :] * 4 + np.arange(4)[None, None, :, None]
           ).reshape(-1)                                                # [NN]
    gk2 = np.take(g8T.view(np.uint8), idx, axis=1)                      # [CD, NN]
    run2 = _get_runner("k2")
    in2 = []
    for c in range(NCORES):
        in2.append({
            "g": _swz(gk2[:, c * NNL:(c + 1) * NNL]).view(F8),
            "hl": _swz(hT8[:, NPL * c:NPL * (c + 1)]),
            "mask": mask,
        })
    res2 = run2(in2)
    sc2 = np.stack([r["sc"] for r in res2])
    cur = (sc2.reshape(NCORES, 128, 2, 4).transpose(0, 2, 1, 3)
           .reshape(NP, K) / np.float32(SC * SC)).astype(np.float32)

    nids = (furthest[:, None] * K + np.arange(K)).reshape(-1)
    lcur = nlab[nids].reshape(NP, K)
    maxcur = np.max(np.where(lcur == 1, cur, -np.inf), axis=1)
    maxp = np.maximum(maxp, maxcur)
    found = ~((maxp == -1.0) | (minp == 1.0))
    lt = np.where(found, maxp - minp + 2.0, 0.0).astype(np.float32)
    not_find = int(np.sum(~found))
    if not_find == NN:
        loss_triple = np.float32(0.0)
    else:
        loss_triple = np.float32(lt.sum(dtype=np.float32) / np.float32(NN - not_find))

    return (np.float32(loss_label), np.float32(loss_norm), np.float32(loss_triple))


# revision 6
# speedup vs baseline: 2.8379x; 1.0442x over previous
"""Trainium2 Bass kernel for nn_ContrastLoss_Disentangle.

Contract: kernel(**inputs) takes the FULL (unsharded) inputs and returns the
same structure the reference returns: (loss_label, loss_norm, loss_triple)
as float32 scalars.

Pipeline (8 NeuronCores, data-parallel):
  host:    norms (exact), normalization, categories folded into nlp rows
           (g = nlpF * cat), everything scaled x16 and cast to fp8_e4m3,
           transposed to [C*D, rows] layouts pre-swizzled for SBUF
  device1: per-core: scores via fp8 DoubleRow PE matmuls (diag-extracted
           with a mask on DVE) + a [512, 1024] block of the pm gram matrix
           (fp8 DoubleRow PE, fp16 output)
  host:    BCE, stable argsort rank-select (furthest), gather of the
           "hard positive" g columns
  device2: per-core: diag-extracted dots of the gathered columns
  host:    triplet loss assembly

All heavy HBM traffic is fp8 (nlp rows: 2 MB/core, pose gram operands:
3 MB/core) and the pm block leaves as fp16, which puts the kernel near the
DMA roofline of the part.  Accuracy: fp8 x16 rounding perturbs scores by
~0.5% absolute and reshuffles `furthest` between near-rank poses; measured
end-to-end max rel err ~3e-5 (gate 2e-2).
"""

import os
import numpy as np
import ml_dtypes

import concourse.bass as bass
import concourse.tile as tile
from concourse import bacc, mybir
from concourse.bass2jax import install_neuronx_cc_hook, partition_id_tensor, _bass_exec_p

C, NP, K, D = 8, 2048, 4, 256
NN = NP * K          # 8192
NCORES = 8
NPL = NP // NCORES   # 256 poses per core
NNL = NN // NCORES   # 1024 nlp rows per core
CD = C * D           # 2048 contraction size
KT = CD // 128       # 16 k-tiles

SC = float(os.environ.get("FP8_SCALE", "16"))
F8 = ml_dtypes.float8_e4m3
DR = os.environ.get("DR", "1") == "1"

# pm block grid: 4 row-blocks x 2 col-blocks
PM_MI, PM_NJ = 4, 2
PM_M = NP // PM_MI   # 512 rows per core block
PM_N = NP // PM_NJ   # 1024 cols per core block

_runners = {}


def _build_kernel(with_pm: bool):
    """Per-core program. Inputs (per core, pre-swizzled so partition p holds
    contraction dims k*128+p and every DMA line is contiguous):
      g    [128, 2, KT, 512] fp8  nlp-side columns, hh-major; col 128*m+p
                                  maps to local nlp row 512*hh+4*p+m
      hl   k1: [128, 2, KT, 256]  pose columns, [own 256 | other 256] of the
                                  pm row-block (rows permuted, see host)
           k2: [128, KT, 256]     own pose columns only
      hr   [128, 2, KT, 512] fp8  pm rhs pose columns, nb-major (k1 only)
      mask [128, 512] bf16        mask[p, 128*m+q] = (q == p)
    Outputs:
      sc   [128, 8] f32           raw dots, col = hh*4 + 2*q + m'
      pm16 [512, 1024] fp16       pm block (k1 only)
    """
    nc = bacc.Bacc("TRN2", target_bir_lowering=False, debug=False,
                   num_devices=NCORES)
    f8 = mybir.dt.float8e4
    if with_pm:
        g_in = nc.dram_tensor("g", [128, 2, KT, 512], f8,
                              kind="ExternalInput").ap()
        hl_in = nc.dram_tensor("hl", [128, 2, KT, 256], f8,
                               kind="ExternalInput").ap()
        hr_in = nc.dram_tensor("hr", [128, 2, KT, 512], f8,
                               kind="ExternalInput").ap()
        pmo = nc.dram_tensor("pm16", [PM_M, PM_N], mybir.dt.float16,
                             kind="ExternalOutput").ap()
    else:
        g_in = nc.dram_tensor("g", [128, 2, KT, 512], f8,
                              kind="ExternalInput").ap()
        hl_in = nc.dram_tensor("hl", [128, KT, 256], f8,
                               kind="ExternalInput").ap()
    mask_in = nc.dram_tensor("mask", [128, 512], mybir.dt.bfloat16,
                             kind="ExternalInput").ap()
    sc_out = nc.dram_tensor("sc", [128, 8], mybir.dt.float32,
                            kind="ExternalOutput").ap()

    with tile.TileContext(nc) as tc:
        with tc.tile_pool(name="big", bufs=1) as big, \
             tc.tile_pool(name="scr", bufs=2) as scr, \
             tc.tile_pool(name="ev", bufs=2) as ev, \
             tc.tile_pool(name="ps", bufs=3, space="PSUM") as ps:

            if with_pm:
                hl_t = big.tile([128, 2, KT, 256], f8, tag="hl")
                g_t = big.tile([128, 2, KT, 512], f8, tag="g")
                hr_t = big.tile([128, 2, KT, 512], f8, tag="hr")
            else:
                hl_t = big.tile([128, KT, 256], f8, tag="hl")
                g_t = big.tile([128, 2, KT, 512], f8, tag="g")
            mask_t = big.tile([128, 512], mybir.dt.bfloat16, tag="mask")
            sc_t = big.tile([128, 8], mybir.dt.float32, tag="sc")

            # ---- DMA stream: critical inputs first ----------------------
            if with_pm:
                nc.sync.dma_start(hl_t[:, 0], hl_in[:, 0])     # own poses
            else:
                nc.sync.dma_start(hl_t[:], hl_in)
            nc.sync.dma_start(g_t[:, 0, 0:8], g_in[:, 0, 0:8])
            nc.sync.dma_start(g_t[:, 0, 8:16], g_in[:, 0, 8:16])
            nc.sync.dma_start(g_t[:, 1, 0:8], g_in[:, 1, 0:8])
            nc.sync.dma_start(g_t[:, 1, 8:16], g_in[:, 1, 8:16])
            nc.sync.dma_start(mask_t[:], mask_in)
            if with_pm:
                nc.sync.dma_start(hl_t[:, 1], hl_in[:, 1])
                nc.sync.dma_start(hr_t[:, 0, 0:8], hr_in[:, 0, 0:8])
                nc.sync.dma_start(hr_t[:, 0, 8:16], hr_in[:, 0, 8:16])
                nc.sync.dma_start(hr_t[:, 1, 0:8], hr_in[:, 1, 0:8])
                nc.sync.dma_start(hr_t[:, 1, 8:16], hr_in[:, 1, 8:16])

            def hl_slice(half, kp, col, w):
                if with_pm:
                    return hl_t[:, half, 2 * kp:2 * kp + 2, col:col + w]
                return hl_t[:, 2 * kp:2 * kp + 2, col:col + w]

            def mm(acc, half, lcol, rt, rsel, rcol, w):
                if DR:
                    for kp in range(KT // 2):
                        nc.tensor.matmul(
                            acc[:], hl_slice(half, kp, lcol, 128),
                            rt[:, rsel, 2 * kp:2 * kp + 2, rcol:rcol + w],
                            start=(kp == 0), stop=(kp == KT // 2 - 1),
                            perf_mode=mybir.MatmulPerfMode.DoubleRow)
                else:
                    for k in range(KT):
                        if with_pm:
                            lh = hl_t[:, half, k, lcol:lcol + 128]
                        else:
                            lh = hl_t[:, k, lcol:lcol + 128]
                        nc.tensor.matmul(
                            acc[:], lh, rt[:, rsel, k, rcol:rcol + w],
                            start=(k == 0), stop=(k == KT - 1))

            # scores: diag blocks of (own poses)^T @ g, quarter psums so the
            # DVE extraction overlaps the next quarter's matmuls
            for hh in range(2):
                for q in range(2):
                    acc_s = ps.tile([128, 512], mybir.dt.float32, tag="ps",
                                    name=f"accs{hh}{q}")
                    mm(acc_s[:, 0:256], 0, 128 * hh, g_t, hh, 256 * q, 256)
                    z = scr.tile([128, 256], mybir.dt.float32, tag="z",
                                 name=f"z{hh}{q}")
                    nc.vector.tensor_tensor(z[:], acc_s[:, 0:256],
                                            mask_t[:, 256 * q:256 * q + 256],
                                            op=mybir.AluOpType.mult)
                    nc.vector.tensor_reduce(
                        sc_t[:, 4 * hh + 2 * q:4 * hh + 2 * q + 2],
                        z[:].rearrange("p (m w) -> p m w", m=2),
                        axis=mybir.AxisListType.X, op=mybir.AluOpType.add)
            nc.sync.dma_start(sc_out, sc_t[:])

            # pm block: hl^T @ hr, batched fp16 evac, one out-DMA per nb
            if with_pm:
                for nb in range(2):
                    o = ev.tile([128, 4, 512], mybir.dt.float16, tag="ev",
                                name=f"ev{nb}")
                    for mb in range(4):
                        acc = ps.tile([128, 512], mybir.dt.float32, tag="ps",
                                      name=f"acc{nb}{mb}")
                        mm(acc, mb // 2, 128 * (mb % 2), hr_t, nb, 0, 512)
                        nc.scalar.copy(o[:, mb], acc[:])
                    nc.sync.dma_start(
                        pmo[:, 512 * nb:512 * (nb + 1)]
                        .rearrange("(mb p) n -> p mb n", p=128), o[:])

    nc.finalize()
    return nc


def _make_runner(nc):
    """Reusable jitted SPMD runner (replicates bass2jax.run_bass_via_pjrt but
    caches the compiled executable across calls)."""
    import jax
    from jax.sharding import Mesh, PartitionSpec
    from jax.experimental.shard_map import shard_map

    install_neuronx_cc_hook()
    partition_name = nc.partition_id_tensor.name if nc.partition_id_tensor else None
    in_names, out_names, out_avals = [], [], []
    for alloc in nc.m.functions[0].allocations:
        if not isinstance(alloc, mybir.MemoryLocationSet):
            continue
        name = alloc.memorylocations[0].name
        if alloc.kind == "ExternalInput":
            if name != partition_name:
                in_names.append(name)
        elif alloc.kind == "ExternalOutput":
            out_names.append(name)
            out_avals.append(jax.core.ShapedArray(
                tuple(alloc.tensor_shape), mybir.dt.np(alloc.dtype)))
    n_params = len(in_names)
    all_in = in_names + out_names + ([partition_name] if partition_name else [])

    def _body(*args):
        operands = list(args)
        if partition_name is not None:
            operands.append(partition_id_tensor())
        outs = _bass_exec_p.bind(
            *operands, out_avals=tuple(out_avals), in_names=tuple(all_in),
            out_names=tuple(out_names), lowering_input_output_aliases=(),
            sim_require_finite=False, sim_require_nnan=False, nc=nc)
        return tuple(outs)

    devices = jax.devices()[:NCORES]
    mesh = Mesh(np.asarray(devices), ("core",))
    donate = tuple(range(n_params, n_params + len(out_names)))
    sharded = jax.jit(
        shard_map(_body, mesh=mesh,
                  in_specs=(PartitionSpec("core"),) * (n_params + len(out_names)),
                  out_specs=(PartitionSpec("core"),) * len(out_names),
                  check_rep=False),
        donate_argnums=donate, keep_unused=True)

    def run(in_maps):
        concat_in = [np.concatenate([np.asarray(m[name]) for m in in_maps], axis=0)
                     for name in in_names]
        zeros = [np.zeros((NCORES * a.shape[0], *a.shape[1:]), a.dtype)
                 for a in out_avals]
        out_arrs = sharded(*concat_in, *zeros)
        return [
            {name: np.asarray(out_arrs[i]).reshape(NCORES, *out_avals[i].shape)[c]
             for i, name in enumerate(out_names)}
            for c in range(NCORES)
        ]

    return run


def _get_runner(key):
    if key not in _runners:
        _runners[key] = _make_runner(_build_kernel(with_pm=(key == "k1")))
    return _runners[key]


def _swz(x):
    """[CD, W] (uint8/fp8) -> [128, KT, W] with partition p holding
    contraction rows k*128+p."""
    return np.ascontiguousarray(x.reshape(KT, 128, x.shape[1]).transpose(1, 0, 2))


def _swz2(x):
    """[CD, 2*W] -> [128, 2, KT, W]: like _swz but with the two column
    halves split out as a leading chunk axis (contiguous DMA chunks)."""
    w = x.shape[1] // 2
    return np.ascontiguousarray(
        x.reshape(KT, 128, 2, w).transpose(1, 2, 0, 3))


def _kernel_host_fallback(inputs):
    """Pure-numpy reference replication, used only if the index tensors do
    not have the canonical arange structure the device layout relies on."""
    nlp = np.asarray(inputs["nlp_features"], np.float32)
    pose = np.asarray(inputs["pose_features"], np.float32)
    nlab = np.asarray(inputs["nlp_label"]).astype(np.int64)
    n2p = np.asarray(inputs["nlpid2poseid"]).astype(np.int64)
    p2n = np.asarray(inputs["pose2nlpid"]).astype(np.int64)
    cat = np.asarray(inputs["categories"], np.float32)
    ri = np.asarray(inputs["rand_index"]).astype(np.int64)
    Np, Nn = pose.shape[1], nlp.shape[1]
    norm_p = np.sqrt(np.einsum("cpd,cpd->cp", pose, pose, dtype=np.float32))
    norm_n = np.sqrt(np.einsum("cnd,cnd->cn", nlp, nlp, dtype=np.float32))
    poseF = pose / norm_p[:, :, None]
    nlpF = nlp / norm_n[:, :, None]
    loss_norm = np.float32(np.float32(norm_p.mean()) + np.float32(norm_n.mean()))
    dots = np.einsum("cnd,cnd->cn", nlpF, poseF[:, n2p]).astype(np.float32)
    scores = np.einsum("cn,nc->n", dots, cat).astype(np.float32)
    p = (1.0 / (1.0 + np.exp(-scores))).astype(np.float32)
    lblf = nlab.astype(np.float32)
    loss_label = np.float32(
        np.mean(-(np.log(p) * lblf + np.log(1.0 - p) * (1.0 - lblf))))
    pf = np.ascontiguousarray(poseF.transpose(0, 2, 1).reshape(-1, Np))
    pm = (pf.T @ pf).astype(np.float32)
    ar = np.arange(Np)
    pm[ar, ar] = 1.0
    order = np.argsort(pm, axis=1, kind="stable")
    furthest = order[ar, ri]
    sg = scores[p2n]
    lg = nlab[p2n]
    maxp = np.maximum(np.max(np.where(lg == 0, sg, -np.inf), axis=1), -1.0)
    minp = np.minimum(np.min(np.where(lg == 1, sg, np.inf), axis=1), 1.0)
    nids = p2n[furthest]
    cd = np.einsum("cpkd,cpd->cpk", nlpF[:, nids], poseF)
    cur = np.einsum("cpk,pkc->pk", cd, cat[nids]).astype(np.float32)
    lcur = nlab[nids]
    maxcur = np.max(np.where(lcur == 1, cur, -np.inf), axis=1)
    maxp = np.maximum(maxp, maxcur)
    found = ~((maxp == -1.0) | (minp == 1.0))
    lt = np.where(found, maxp - minp + 2.0, 0.0).astype(np.float32)
    nf = int(np.sum(~found))
    loss_triple = (np.float32(0.0) if nf == Nn else
                   np.float32(lt.sum(dtype=np.float32) / np.float32(Nn - nf)))
    return (np.float32(loss_label), loss_norm, loss_triple)


def kernel(**inputs):
    nlp = np.ascontiguousarray(inputs["nlp_features"], np.float32)      # [C, NN, D]
    pose = np.ascontiguousarray(inputs["pose_features"], np.float32)    # [C, NP, D]
    nlab = np.asarray(inputs["nlp_label"]).astype(np.int64)
    cat = np.ascontiguousarray(inputs["categories"], np.float32)        # [NN, C]
    ri = np.asarray(inputs["rand_index"]).astype(np.int64)

    n2p = np.asarray(inputs["nlpid2poseid"]).astype(np.int64)
    p2n = np.asarray(inputs["pose2nlpid"]).astype(np.int64)
    if (not np.array_equal(n2p, np.arange(NN) // K)
            or not np.array_equal(p2n, np.arange(NN).reshape(NP, K))):
        return _kernel_host_fallback(inputs)

    # ---- host: exact norms, normalize, fold categories, fp8 x16 ---------
    norm_p = np.sqrt(np.einsum("cpd,cpd->cp", pose, pose, dtype=np.float32,
                               optimize=True)).astype(np.float32)       # [C, NP]
    norm_n = np.sqrt(np.einsum("cnd,cnd->cn", nlp, nlp, dtype=np.float32,
                               optimize=True)).astype(np.float32)       # [C, NN]
    loss_norm = np.float32(np.float32(norm_p.mean()) + np.float32(norm_n.mean()))

    poseF = pose / norm_p[:, :, None]
    hT8 = np.ascontiguousarray(
        (poseF * SC).transpose(0, 2, 1)).reshape(CD, NP).astype(F8)     # [CD, NP]

    gscale = (cat.T / norm_n) * SC                                      # [C, NN]
    g8 = (nlp * gscale[:, :, None]).astype(F8)                          # [C, NN, D]
    g8T = np.ascontiguousarray(g8.transpose(0, 2, 1)).reshape(CD, NN)   # [CD, NN]
    # (p, m) -> (m, p) within each 512-column block so the device's diag
    # mask lines up: col 512*hh + 128*m + p <- local row 512*hh + 4*p + m
    g8km = np.ascontiguousarray(
        g8T.reshape(CD, NN // 512, 128, 4).transpose(0, 1, 3, 2)
    ).reshape(CD, NN)

    mask = np.zeros((128, 512), np.float32)
    mask[np.arange(128)[:, None], 128 * np.arange(4)[None, :] + np.arange(128)[:, None]] = 1.0
    mask = mask.astype(ml_dtypes.bfloat16)

    # ---- device kernel 1 -------------------------------------------------
    # hl column order per core: own 256 poses first (so the scores matmul
    # can address them at a fixed offset), then the other half of the pm
    # row-block; pm rows are written back through rows_order.
    run1 = _get_runner("k1")
    in1 = []
    rows_order = np.empty((NCORES, PM_M), np.int64)
    for c in range(NCORES):
        i, j = c // PM_NJ, c % PM_NJ
        par = c % 2
        own = np.arange(512 * i + 256 * par, 512 * i + 256 * par + 256)
        oth = np.arange(512 * i + 256 * (1 - par), 512 * i + 256 * (1 - par) + 256)
        cols = np.concatenate([own, oth])
        rows_order[c] = cols
        in1.append({
            "g": _swz2(g8km[:, c * NNL:(c + 1) * NNL]),
            "hl": _swz2(hT8[:, cols]),
            "hr": _swz2(hT8[:, PM_N * j:PM_N * (j + 1)]),
            "mask": mask,
        })
    res1 = run1(in1)

    # ---- host: scores / BCE ---------------------------------------------
    sc_all = np.stack([r["sc"] for r in res1])                          # [8,128,8]
    scores = (sc_all.reshape(NCORES, 128, 2, 4).transpose(0, 2, 1, 3)
              .reshape(NN) / np.float32(SC * SC)).astype(np.float32)
    p = (1.0 / (1.0 + np.exp(-scores))).astype(np.float32)
    lblf = nlab.astype(np.float32)
    loss_label = np.float32(
        np.mean(-(np.log(p) * lblf + np.log(1.0 - p) * (1.0 - lblf))))

    # ---- host: furthest selection ---------------------------------------
    pm = np.empty((NP, NP), np.float32)
    for c in range(NCORES):
        j = c % PM_NJ
        pm[rows_order[c], PM_N * j:PM_N * (j + 1)] = \
            res1[c]["pm16"].astype(np.float32)
    ar = np.arange(NP)
    pm[ar, ar] = np.float32(SC * SC)
    order = np.argsort(pm, axis=1, kind="stable")
    furthest = order[ar, ri]                                            # [NP]

    sg = scores.reshape(NP, K)
    lg = nlab.reshape(NP, K)
    maxp = np.maximum(np.max(np.where(lg == 0, sg, -np.inf), axis=1), -1.0)
    minp = np.minimum(np.min(np.where(lg == 1, sg, np.inf), axis=1), 1.0)

    # ---- device kernel 2: dots of gathered hard-positive columns ---------
    fq = furthest.reshape(NCORES, 2, 128)
    idx = (fq[:, :, None, :] * 4 + np.arange(4)[None, None, :, None]
           ).reshape(-1)                                                # [NN]
    gk2 = np.take(g8T.view(np.uint8), idx, axis=1)                      # [CD, NN]
    run2 = _get_runner("k2")
    in2 = []
    for c in range(NCORES):
        in2.append({
            "g": _swz2(gk2[:, c * NNL:(c + 1) * NNL]).view(F8),
            "hl": _swz(hT8[:, NPL * c:NPL * (c + 1)]),
            "mask": mask,
        })
    res2 = run2(in2)
    sc2 = np.stack([r["sc"] for r in res2])
    cur = (sc2.reshape(NCORES, 128, 2, 4).transpose(0, 2, 1, 3)
           .reshape(NP, K) / np.float32(SC * SC)).astype(np.float32)

    nids = (furthest[:, None] * K + np.arange(K)).reshape(-1)
    lcur = nlab[nids].reshape(NP, K)
    maxcur = np.max(np.where(lcur == 1, cur, -np.inf), axis=1)
    maxp = np.maximum(maxp, maxcur)
    found = ~((maxp == -1.0) | (minp == 1.0))
    lt = np.where(found, maxp - minp + 2.0, 0.0).astype(np.float32)
    not_find = int(np.sum(~found))
    if not_find == NN:
        loss_triple = np.float32(0.0)
    else:
        loss_triple = np.float32(lt.sum(dtype=np.float32) / np.float32(NN - not_find))

    return (np.float32(loss_label), np.float32(loss_norm), np.float32(loss_triple))


# revision 12
# speedup vs baseline: 2.9796x; 1.0499x over previous
"""Trainium2 Bass kernel for nn_ContrastLoss_Disentangle.

Contract: kernel(**inputs) takes the FULL (unsharded) inputs and returns the
same structure the reference returns: (loss_label, loss_norm, loss_triple)
as float32 scalars.

Pipeline (8 NeuronCores, data-parallel):
  host:    norms (exact), normalization, categories folded into nlp rows
           (g = nlpF * cat), everything scaled x16 and cast to fp8_e4m3,
           transposed to [C*D, rows] layouts pre-swizzled for SBUF
  device1: per-core: scores via fp8 DoubleRow PE matmuls (diag-extracted
           with a mask on DVE) + a [512, 1024] block of the pm gram matrix
           (fp8 DoubleRow PE, fp16 output)
  host:    BCE, stable argsort rank-select (furthest), gather of the
           "hard positive" g columns
  device2: per-core: diag-extracted dots of the gathered columns
  host:    triplet loss assembly

All heavy HBM traffic is fp8 (nlp rows: 2 MB/core, pose gram operands:
3 MB/core) and the pm block leaves as fp16, which puts the kernel near the
DMA roofline of the part.  Accuracy: fp8 x16 rounding perturbs scores by
~0.5% absolute and reshuffles `furthest` between near-rank poses; measured
end-to-end max rel err ~3e-5 (gate 2e-2).
"""

import os
import numpy as np
import ml_dtypes

import concourse.bass as bass
import concourse.tile as tile
from concourse import bacc, mybir
from concourse.bass2jax import install_neuronx_cc_hook, partition_id_tensor, _bass_exec_p

C, NP, K, D = 8, 2048, 4, 256
NN = NP * K          # 8192
NCORES = 8
NPL = NP // NCORES   # 256 poses per core
NNL = NN // NCORES   # 1024 nlp rows per core
CD = C * D           # 2048 contraction size
KT = CD // 128       # 16 k-tiles

SC = float(os.environ.get("FP8_SCALE", "16"))
F8 = ml_dtypes.float8_e4m3
DR = os.environ.get("DR", "1") == "1"

# pm block grid: 4 row-blocks x 2 col-blocks
PM_MI, PM_NJ = 4, 2
PM_M = NP // PM_MI   # 512 rows per core block
PM_N = NP // PM_NJ   # 1024 cols per core block

_runners = {}


def _build_kernel(with_pm: bool):
    """Per-core program. Inputs (per core, pre-swizzled so partition p holds
    contraction dims k*128+p and every DMA line is contiguous):
      g    [128, 2, KT, 512] fp8  nlp-side columns, hh-major; col 128*m+p
                                  maps to local nlp row 512*hh+4*p+m
      hl   k1: [128, 2, KT, 256]  pose columns, [own 256 | other 256] of the
                                  pm row-block (rows permuted, see host)
           k2: [128, KT, 256]     own pose columns only
      hr   [128, 2, KT, 512] fp8  pm rhs pose columns, nb-major (k1 only)
      mask [128, 512] bf16        mask[p, 128*m+q] = (q == p)
    Outputs:
      sc   [128, 8] f32           raw dots, col = hh*4 + 2*q + m'
      pm16 [512, 1024] fp16       pm block (k1 only)
    """
    nc = bacc.Bacc("TRN2", target_bir_lowering=False, debug=False,
                   num_devices=NCORES)
    f8 = mybir.dt.float8e4
    if with_pm:
        g_in = nc.dram_tensor("g", [128, 2, KT, 512], f8,
                              kind="ExternalInput").ap()
        hl_in = nc.dram_tensor("hl", [128, 2, KT, 256], f8,
                               kind="ExternalInput").ap()
        hr_in = nc.dram_tensor("hr", [128, 2, KT, 512], f8,
                               kind="ExternalInput").ap()
        pmo = nc.dram_tensor("pm16", [PM_M, PM_N], mybir.dt.float16,
                             kind="ExternalOutput").ap()
    else:
        g_in = nc.dram_tensor("g", [128, 2, KT, 512], f8,
                              kind="ExternalInput").ap()
        hl_in = nc.dram_tensor("hl", [128, KT, 256], f8,
                               kind="ExternalInput").ap()
    mask_in = nc.dram_tensor("mask", [128, 512], mybir.dt.bfloat16,
                             kind="ExternalInput").ap()
    sc_out = nc.dram_tensor("sc", [128, 8], mybir.dt.float32,
                            kind="ExternalOutput").ap()

    with tile.TileContext(nc) as tc:
        with tc.tile_pool(name="big", bufs=1) as big, \
             tc.tile_pool(name="scr", bufs=4) as scr, \
             tc.tile_pool(name="ev", bufs=2) as ev, \
             tc.tile_pool(name="ps", bufs=4, space="PSUM") as ps:

            if with_pm:
                hl_t = big.tile([128, 2, KT, 256], f8, tag="hl")
                g_t = big.tile([128, 2, KT, 512], f8, tag="g")
                hr_t = big.tile([128, 2, KT, 512], f8, tag="hr")
            else:
                hl_t = big.tile([128, KT, 256], f8, tag="hl")
                g_t = big.tile([128, 2, KT, 512], f8, tag="g")
            mask_t = big.tile([128, 512], mybir.dt.bfloat16, tag="mask")
            sc_t = big.tile([128, 8], mybir.dt.float32, tag="sc")

            # ---- DMA stream: critical inputs first, mask early (it gates
            # the DVE/Pool extractions and psum-slot reuse) ----------------
            if with_pm:
                nc.sync.dma_start(hl_t[:, 0], hl_in[:, 0])     # own poses
            else:
                nc.sync.dma_start(hl_t[:], hl_in)
            nc.sync.dma_start(mask_t[:], mask_in)
            nc.sync.dma_start(g_t[:, 0, 0:8], g_in[:, 0, 0:8])
            nc.sync.dma_start(g_t[:, 0, 8:16], g_in[:, 0, 8:16])
            nc.sync.dma_start(g_t[:, 1, 0:8], g_in[:, 1, 0:8])
            nc.sync.dma_start(g_t[:, 1, 8:16], g_in[:, 1, 8:16])
            if with_pm:
                nc.sync.dma_start(hl_t[:, 1], hl_in[:, 1])
                nc.sync.dma_start(hr_t[:, 0, 0:8], hr_in[:, 0, 0:8])
                nc.sync.dma_start(hr_t[:, 0, 8:16], hr_in[:, 0, 8:16])
                nc.sync.dma_start(hr_t[:, 1, 0:8], hr_in[:, 1, 0:8])
                nc.sync.dma_start(hr_t[:, 1, 8:16], hr_in[:, 1, 8:16])

            def hl_slice(half, kp, col, w):
                if with_pm:
                    return hl_t[:, half, 2 * kp:2 * kp + 2, col:col + w]
                return hl_t[:, 2 * kp:2 * kp + 2, col:col + w]

            def mm(acc, half, lcol, rt, rsel, rcol, w):
                if DR:
                    for kp in range(KT // 2):
                        nc.tensor.matmul(
                            acc[:], hl_slice(half, kp, lcol, 128),
                            rt[:, rsel, 2 * kp:2 * kp + 2, rcol:rcol + w],
                            start=(kp == 0), stop=(kp == KT // 2 - 1),
                            perf_mode=mybir.MatmulPerfMode.DoubleRow)
                else:
                    for k in range(KT):
                        if with_pm:
                            lh = hl_t[:, half, k, lcol:lcol + 128]
                        else:
                            lh = hl_t[:, k, lcol:lcol + 128]
                        nc.tensor.matmul(
                            acc[:], lh, rt[:, rsel, k, rcol:rcol + w],
                            start=(k == 0), stop=(k == KT - 1))

            # scores: diag blocks of (own poses)^T @ g, quarter psums; the
            # two quarters of each half extract on different engines
            # (DVE / GpSimd) so the extractions run in parallel
            for hh in range(2):
                for q in range(2):
                    acc_s = ps.tile([128, 512], mybir.dt.float32, tag="ps",
                                    name=f"accs{hh}{q}")
                    mm(acc_s[:, 0:256], 0, 128 * hh, g_t, hh, 256 * q, 256)
                    z = scr.tile([128, 256], mybir.dt.float32, tag="z",
                                 name=f"z{hh}{q}")
                    if q == 0:
                        nc.vector.tensor_tensor(z[:], acc_s[:, 0:256],
                                                mask_t[:, 0:256],
                                                op=mybir.AluOpType.mult)
                        nc.vector.tensor_reduce(
                            sc_t[:, 4 * hh:4 * hh + 2],
                            z[:].rearrange("p (m w) -> p m w", m=2),
                            axis=mybir.AxisListType.X, op=mybir.AluOpType.add)
                    else:
                        nc.vector.tensor_tensor(z[:], acc_s[:, 0:256],
                                                mask_t[:, 256:512],
                                                op=mybir.AluOpType.mult)
                        zd = scr.tile([128, 128], mybir.dt.float32, tag="zd",
                                      name=f"zd{hh}")
                        for m in range(2):
                            nc.scalar.activation(
                                zd[:], z[:, 128 * m:128 * m + 128],
                                mybir.ActivationFunctionType.Copy,
                                accum_out=sc_t[:, 4 * hh + 2 + m:
                                               4 * hh + 3 + m])
            nc.sync.dma_start(sc_out, sc_t[:])

            # pm block: hl^T @ hr; evacs alternate ACT/DVE, outputs leave in
            # 2-row-block chunks so the tail chain stays short
            if with_pm:
                for nb in range(2):
                    o = ev.tile([128, 4, 512], mybir.dt.float16, tag="ev",
                                name=f"ev{nb}")
                    for mb in range(4):
                        acc = ps.tile([128, 512], mybir.dt.float32, tag="ps",
                                      name=f"acc{nb}{mb}")
                        mm(acc, mb // 2, 128 * (mb % 2), hr_t, nb, 0, 512)
                        if mb % 2 == 0:
                            nc.scalar.copy(o[:, mb], acc[:])
                        else:
                            nc.vector.tensor_scalar_add(o[:, mb], acc[:], 0.0)
                        if mb % 2 == 1:
                            nc.sync.dma_start(
                                pmo[:, 512 * nb:512 * (nb + 1)]
                                .rearrange("(mb p) n -> p mb n", p=128)
                                [:, mb - 1:mb + 1], o[:, mb - 1:mb + 1])

    nc.finalize()
    return nc


def _make_runner(nc):
    """Reusable jitted SPMD runner (replicates bass2jax.run_bass_via_pjrt but
    caches the compiled executable across calls)."""
    import jax
    from jax.sharding import Mesh, PartitionSpec
    from jax.experimental.shard_map import shard_map

    install_neuronx_cc_hook()
    partition_name = nc.partition_id_tensor.name if nc.partition_id_tensor else None
    in_names, out_names, out_avals = [], [], []
    for alloc in nc.m.functions[0].allocations:
        if not isinstance(alloc, mybir.MemoryLocationSet):
            continue
        name = alloc.memorylocations[0].name
        if alloc.kind == "ExternalInput":
            if name != partition_name:
                in_names.append(name)
        elif alloc.kind == "ExternalOutput":
            out_names.append(name)
            out_avals.append(jax.core.ShapedArray(
                tuple(alloc.tensor_shape), mybir.dt.np(alloc.dtype)))
    n_params = len(in_names)
    all_in = in_names + out_names + ([partition_name] if partition_name else [])

    def _body(*args):
        operands = list(args)
        if partition_name is not None:
            operands.append(partition_id_tensor())
        outs = _bass_exec_p.bind(
            *operands, out_avals=tuple(out_avals), in_names=tuple(all_in),
            out_names=tuple(out_names), lowering_input_output_aliases=(),
            sim_require_finite=False, sim_require_nnan=False, nc=nc)
        return tuple(outs)

    devices = jax.devices()[:NCORES]
    mesh = Mesh(np.asarray(devices), ("core",))
    donate = tuple(range(n_params, n_params + len(out_names)))
    sharded = jax.jit(
        shard_map(_body, mesh=mesh,
                  in_specs=(PartitionSpec("core"),) * (n_params + len(out_names)),
                  out_specs=(PartitionSpec("core"),) * len(out_names),
                  check_rep=False),
        donate_argnums=donate, keep_unused=True)

    def run(in_maps):
        concat_in = [np.concatenate([np.asarray(m[name]) for m in in_maps], axis=0)
                     for name in in_names]
        zeros = [np.zeros((NCORES * a.shape[0], *a.shape[1:]), a.dtype)
                 for a in out_avals]
        out_arrs = sharded(*concat_in, *zeros)
        return [
            {name: np.asarray(out_arrs[i]).reshape(NCORES, *out_avals[i].shape)[c]
             for i, name in enumerate(out_names)}
            for c in range(NCORES)
        ]

    return run


def _get_runner(key):
    if key not in _runners:
        _runners[key] = _make_runner(_build_kernel(with_pm=(key == "k1")))
    return _runners[key]


def _swz(x):
    """[CD, W] (uint8/fp8) -> [128, KT, W] with partition p holding
    contraction rows k*128+p."""
    return np.ascontiguousarray(x.reshape(KT, 128, x.shape[1]).transpose(1, 0, 2))


def _swz2(x):
    """[CD, 2*W] -> [128, 2, KT, W]: like _swz but with the two column
    halves split out as a leading chunk axis (contiguous DMA chunks)."""
    w = x.shape[1] // 2
    return np.ascontiguousarray(
        x.reshape(KT, 128, 2, w).transpose(1, 2, 0, 3))


def _kernel_host_fallback(inputs):
    """Pure-numpy reference replication, used only if the index tensors do
    not have the canonical arange structure the device layout relies on."""
    nlp = np.asarray(inputs["nlp_features"], np.float32)
    pose = np.asarray(inputs["pose_features"], np.float32)
    nlab = np.asarray(inputs["nlp_label"]).astype(np.int64)
    n2p = np.asarray(inputs["nlpid2poseid"]).astype(np.int64)
    p2n = np.asarray(inputs["pose2nlpid"]).astype(np.int64)
    cat = np.asarray(inputs["categories"], np.float32)
    ri = np.asarray(inputs["rand_index"]).astype(np.int64)
    Np, Nn = pose.shape[1], nlp.shape[1]
    norm_p = np.sqrt(np.einsum("cpd,cpd->cp", pose, pose, dtype=np.float32))
    norm_n = np.sqrt(np.einsum("cnd,cnd->cn", nlp, nlp, dtype=np.float32))
    poseF = pose / norm_p[:, :, None]
    nlpF = nlp / norm_n[:, :, None]
    loss_norm = np.float32(np.float32(norm_p.mean()) + np.float32(norm_n.mean()))
    dots = np.einsum("cnd,cnd->cn", nlpF, poseF[:, n2p]).astype(np.float32)
    scores = np.einsum("cn,nc->n", dots, cat).astype(np.float32)
    p = (1.0 / (1.0 + np.exp(-scores))).astype(np.float32)
    lblf = nlab.astype(np.float32)
    loss_label = np.float32(
        np.mean(-(np.log(p) * lblf + np.log(1.0 - p) * (1.0 - lblf))))
    pf = np.ascontiguousarray(poseF.transpose(0, 2, 1).reshape(-1, Np))
    pm = (pf.T @ pf).astype(np.float32)
    ar = np.arange(Np)
    pm[ar, ar] = 1.0
    order = np.argsort(pm, axis=1, kind="stable")
    furthest = order[ar, ri]
    sg = scores[p2n]
    lg = nlab[p2n]
    maxp = np.maximum(np.max(np.where(lg == 0, sg, -np.inf), axis=1), -1.0)
    minp = np.minimum(np.min(np.where(lg == 1, sg, np.inf), axis=1), 1.0)
    nids = p2n[furthest]
    cd = np.einsum("cpkd,cpd->cpk", nlpF[:, nids], poseF)
    cur = np.einsum("cpk,pkc->pk", cd, cat[nids]).astype(np.float32)
    lcur = nlab[nids]
    maxcur = np.max(np.where(lcur == 1, cur, -np.inf), axis=1)
    maxp = np.maximum(maxp, maxcur)
    found = ~((maxp == -1.0) | (minp == 1.0))
    lt = np.where(found, maxp - minp + 2.0, 0.0).astype(np.float32)
    nf = int(np.sum(~found))
    loss_triple = (np.float32(0.0) if nf == Nn else
                   np.float32(lt.sum(dtype=np.float32) / np.float32(Nn - nf)))
    return (np.float32(loss_label), loss_norm, loss_triple)


def kernel(**inputs):
    nlp = np.ascontiguousarray(inputs["nlp_features"], np.float32)      # [C, NN, D]
    pose = np.ascontiguousarray(inputs["pose_features"], np.float32)    # [C, NP, D]
    nlab = np.asarray(inputs["nlp_label"]).astype(np.int64)
    cat = np.ascontiguousarray(inputs["categories"], np.float32)        # [NN, C]
    ri = np.asarray(inputs["rand_index"]).astype(np.int64)

    n2p = np.asarray(inputs["nlpid2poseid"]).astype(np.int64)
    p2n = np.asarray(inputs["pose2nlpid"]).astype(np.int64)
    if (not np.array_equal(n2p, np.arange(NN) // K)
            or not np.array_equal(p2n, np.arange(NN).reshape(NP, K))):
        return _kernel_host_fallback(inputs)

    # ---- host: exact norms, normalize, fold categories, fp8 x16 ---------
    norm_p = np.sqrt(np.einsum("cpd,cpd->cp", pose, pose, dtype=np.float32,
                               optimize=True)).astype(np.float32)       # [C, NP]
    norm_n = np.sqrt(np.einsum("cnd,cnd->cn", nlp, nlp, dtype=np.float32,
                               optimize=True)).astype(np.float32)       # [C, NN]
    loss_norm = np.float32(np.float32(norm_p.mean()) + np.float32(norm_n.mean()))

    poseF = pose / norm_p[:, :, None]
    hT8 = np.ascontiguousarray(
        (poseF * SC).transpose(0, 2, 1)).reshape(CD, NP).astype(F8)     # [CD, NP]

    gscale = (cat.T / norm_n) * SC                                      # [C, NN]
    g8 = (nlp * gscale[:, :, None]).astype(F8)                          # [C, NN, D]
    g8T = np.ascontiguousarray(g8.transpose(0, 2, 1)).reshape(CD, NN)   # [CD, NN]
    # (p, m) -> (m, p) within each 512-column block so the device's diag
    # mask lines up: col 512*hh + 128*m + p <- local row 512*hh + 4*p + m
    g8km = np.ascontiguousarray(
        g8T.reshape(CD, NN // 512, 128, 4).transpose(0, 1, 3, 2)
    ).reshape(CD, NN)

    mask = np.zeros((128, 512), np.float32)
    mask[np.arange(128)[:, None], 128 * np.arange(4)[None, :] + np.arange(128)[:, None]] = 1.0
    mask = mask.astype(ml_dtypes.bfloat16)

    # ---- device kernel 1 -------------------------------------------------
    # hl column order per core: own 256 poses first (so the scores matmul
    # can address them at a fixed offset), then the other half of the pm
    # row-block; pm rows are written back through rows_order.
    run1 = _get_runner("k1")
    in1 = []
    rows_order = np.empty((NCORES, PM_M), np.int64)
    for c in range(NCORES):
        i, j = c // PM_NJ, c % PM_NJ
        par = c % 2
        own = np.arange(512 * i + 256 * par, 512 * i + 256 * par + 256)
        oth = np.arange(512 * i + 256 * (1 - par), 512 * i + 256 * (1 - par) + 256)
        cols = np.concatenate([own, oth])
        rows_order[c] = cols
        in1.append({
            "g": _swz2(g8km[:, c * NNL:(c + 1) * NNL]),
            "hl": _swz2(hT8[:, cols]),
            "hr": _swz2(hT8[:, PM_N * j:PM_N * (j + 1)]),
            "mask": mask,
        })
    res1 = run1(in1)

    # ---- host: scores / BCE ---------------------------------------------
    sc_all = np.stack([r["sc"] for r in res1])                          # [8,128,8]
    scores = (sc_all.reshape(NCORES, 128, 2, 4).transpose(0, 2, 1, 3)
              .reshape(NN) / np.float32(SC * SC)).astype(np.float32)
    p = (1.0 / (1.0 + np.exp(-scores))).astype(np.float32)
    lblf = nlab.astype(np.float32)
    loss_label = np.float32(
        np.mean(-(np.log(p) * lblf + np.log(1.0 - p) * (1.0 - lblf))))

    # ---- host: furthest selection ---------------------------------------
    pm = np.empty((NP, NP), np.float32)
    for c in range(NCORES):
        j = c % PM_NJ
        pm[rows_order[c], PM_N * j:PM_N * (j + 1)] = \
            res1[c]["pm16"].astype(np.float32)
    ar = np.arange(NP)
    pm[ar, ar] = np.float32(SC * SC)
    order = np.argsort(pm, axis=1, kind="stable")
    furthest = order[ar, ri]                                            # [NP]

    sg = scores.reshape(NP, K)
    lg = nlab.reshape(NP, K)
    maxp = np.maximum(np.max(np.where(lg == 0, sg, -np.inf), axis=1), -1.0)
    minp = np.minimum(np.min(np.where(lg == 1, sg, np.inf), axis=1), 1.0)

    # ---- device kernel 2: dots of gathered hard-positive columns ---------
    fq = furthest.reshape(NCORES, 2, 128)
    idx = (fq[:, :, None, :] * 4 + np.arange(4)[None, None, :, None]
           ).reshape(-1)                                                # [NN]
    gk2 = np.take(g8T.view(np.uint8), idx, axis=1)                      # [CD, NN]
    run2 = _get_runner("k2")
    in2 = []
    for c in range(NCORES):
        in2.append({
            "g": _swz2(gk2[:, c * NNL:(c + 1) * NNL]).view(F8),
            "hl": _swz(hT8[:, NPL * c:NPL * (c + 1)]),
            "mask": mask,
        })
    res2 = run2(in2)
    sc2 = np.stack([r["sc"] for r in res2])
    cur = (sc2.reshape(NCORES, 128, 2, 4).transpose(0, 2, 1, 3)
           .reshape(NP, K) / np.float32(SC * SC)).astype(np.float32)

    nids = (furthest[:, None] * K + np.arange(K)).reshape(-1)
    lcur = nlab[nids].reshape(NP, K)
    maxcur = np.max(np.where(lcur == 1, cur, -np.inf), axis=1)
    maxp = np.maximum(maxp, maxcur)
    found = ~((maxp == -1.0) | (minp == 1.0))
    lt = np.where(found, maxp - minp + 2.0, 0.0).astype(np.float32)
    not_find = int(np.sum(~found))
    if not_find == NN:
        loss_triple = np.float32(0.0)
    else:
        loss_triple = np.float32(lt.sum(dtype=np.float32) / np.float32(NN - not_find))

    return (np.float32(loss_label), np.float32(loss_norm), np.float32(loss_triple))


# revision 13
# speedup vs baseline: 3.2270x; 1.0830x over previous
"""Trainium2 Bass kernel for nn_ContrastLoss_Disentangle.

Contract: kernel(**inputs) takes the FULL (unsharded) inputs and returns the
same structure the reference returns: (loss_label, loss_norm, loss_triple)
as float32 scalars.

Pipeline (8 NeuronCores, data-parallel):
  host:    norms (exact), normalization, categories folded into nlp rows
           (g = nlpF * cat), everything scaled x16 and cast to fp8_e4m3,
           transposed to [C*D, rows] layouts pre-swizzled for SBUF
  device1: per-core: scores via fp8 DoubleRow PE matmuls (diag-extracted
           with a mask on DVE/ACT) + a [512, 1024] block of the pm gram
           matrix (fp8 DoubleRow PE, descaled fp8 output)
  host:    BCE, stable argsort rank-select (furthest), gather+pack of the
           label-1 "hard positive" g columns
  device2: per-core: dots of the packed columns, additive -1e9 mask and
           reduce-max -> per-pose maxcur directly
  host:    triplet loss assembly

All heavy HBM traffic is fp8 (nlp rows: 2 MB/core, pose gram operands:
3 MB/core, packed hard-positives: ~1.3 MB/core) which puts both kernels
near the serialized-DMA roofline of the part.  Accuracy: fp8 x16 rounding
perturbs scores by ~0.5% absolute and reshuffles `furthest` between
near-rank poses; measured end-to-end max rel err ~8e-4 (gate 2e-2).
"""

import os
import numpy as np
import ml_dtypes

import concourse.bass as bass
import concourse.tile as tile
from concourse import bacc, mybir
from concourse.bass2jax import install_neuronx_cc_hook, partition_id_tensor, _bass_exec_p

C, NP, K, D = 8, 2048, 4, 256
NN = NP * K          # 8192
NCORES = 8
NPL = NP // NCORES   # 256 poses per core
NNL = NN // NCORES   # 1024 nlp rows per core
CD = C * D           # 2048 contraction size
KT = CD // 128       # 16 k-tiles

SC = float(os.environ.get("FP8_SCALE", "16"))
F8 = ml_dtypes.float8_e4m3
DR = os.environ.get("DR", "1") == "1"
W2 = 320             # packed label-1 columns per 128-pose half (mean 256)
NEG = -1.0e9

# pm block grid: 4 row-blocks x 2 col-blocks
PM_MI, PM_NJ = 4, 2
PM_M = NP // PM_MI   # 512 rows per core block
PM_N = NP // PM_NJ   # 1024 cols per core block

_runners = {}


def _build_kernel(with_pm: bool):
    """Per-core program. Inputs (per core, pre-swizzled so partition p holds
    contraction dims k*128+p and every DMA line is contiguous):
      k1 (with_pm):
        g    [128, 2, KT, 512] fp8  nlp-side columns, hh-major; col 128*m+p
                                    maps to local nlp row 512*hh+4*p+m
        hl   [128, 2, KT, 256] fp8  pose columns, [own 256 | other 256] of
                                    the pm row-block (rows permuted on host)
        hr   [128, 2, KT, 512] fp8  pm rhs pose columns, nb-major
        mask [128, 512] bf16        mask[p, 128*m+q] = (q == p)
        outs: sc [128, 8] f32 (col = hh*4 + m);  pm8 [512, 1024] fp8 (/256)
      k2 (packed hard-positives):
        g    [128, 2, KT, W2] fp8   packed label-1 columns per half
        hl   [128, KT, 256] fp8     own pose columns
        maskn [128, 2*W2] bf16      0 where column belongs to pose p, -1e9
                                    elsewhere (incl. padding)
        outs: mx [128, 2] f32       per-pose max of masked dots
    """
    nc = bacc.Bacc("TRN2", target_bir_lowering=False, debug=False,
                   num_devices=NCORES)
    f8 = mybir.dt.float8e4
    if with_pm:
        g_in = nc.dram_tensor("g", [128, 2, KT, 512], f8,
                              kind="ExternalInput").ap()
        hl_in = nc.dram_tensor("hl", [128, 2, KT, 256], f8,
                               kind="ExternalInput").ap()
        hr_in = nc.dram_tensor("hr", [128, 2, KT, 512], f8,
                               kind="ExternalInput").ap()
        mask_in = nc.dram_tensor("mask", [128, 512], mybir.dt.bfloat16,
                                 kind="ExternalInput").ap()
        pmo = nc.dram_tensor("pm8", [PM_M, PM_N], f8,
                             kind="ExternalOutput").ap()
        sc_out = nc.dram_tensor("sc", [128, 8], mybir.dt.float32,
                                kind="ExternalOutput").ap()
    else:
        g_in = nc.dram_tensor("g", [128, 2, KT, W2], f8,
                              kind="ExternalInput").ap()
        hl_in = nc.dram_tensor("hl", [128, KT, 256], f8,
                               kind="ExternalInput").ap()
        mask_in = nc.dram_tensor("maskn", [128, 2 * W2], mybir.dt.bfloat16,
                                 kind="ExternalInput").ap()
        mx_out = nc.dram_tensor("mx", [128, 2], mybir.dt.float32,
                                kind="ExternalOutput").ap()

    with tile.TileContext(nc) as tc:
        with tc.tile_pool(name="big", bufs=1) as big, \
             tc.tile_pool(name="scr", bufs=4) as scr, \
             tc.tile_pool(name="ev", bufs=2) as ev, \
             tc.tile_pool(name="ps", bufs=4, space="PSUM") as ps:

            if with_pm:
                hl_t = big.tile([128, 2, KT, 256], f8, tag="hl")
                g_t = big.tile([128, 2, KT, 512], f8, tag="g")
                hr_t = big.tile([128, 2, KT, 512], f8, tag="hr")
                mask_t = big.tile([128, 512], mybir.dt.bfloat16, tag="mask")
                sc_t = big.tile([128, 8], mybir.dt.float32, tag="sc")
            else:
                hl_t = big.tile([128, KT, 256], f8, tag="hl")
                g_t = big.tile([128, 2, KT, W2], f8, tag="g")
                mask_t = big.tile([128, 2 * W2], mybir.dt.bfloat16,
                                  tag="mask")
                mx_t = big.tile([128, 2], mybir.dt.float32, tag="mx")

            # ---- DMA stream: critical inputs first, mask early (it gates
            # the extractions and psum-slot reuse) --------------------------
            if with_pm:
                nc.sync.dma_start(hl_t[:, 0], hl_in[:, 0])     # own poses
            else:
                nc.sync.dma_start(hl_t[:], hl_in)
            nc.sync.dma_start(mask_t[:], mask_in)
            for hh in range(2):
                nc.sync.dma_start(g_t[:, hh, 0:8], g_in[:, hh, 0:8])
                nc.sync.dma_start(g_t[:, hh, 8:16], g_in[:, hh, 8:16])
            if with_pm:
                nc.sync.dma_start(hl_t[:, 1], hl_in[:, 1])
                for nb in range(2):
                    for kq in range(4):
                        nc.sync.dma_start(
                            hr_t[:, nb, 4 * kq:4 * (kq + 1)],
                            hr_in[:, nb, 4 * kq:4 * (kq + 1)])

            def hl_slice(half, kp, col, w):
                if with_pm:
                    return hl_t[:, half, 2 * kp:2 * kp + 2, col:col + w]
                return hl_t[:, 2 * kp:2 * kp + 2, col:col + w]

            def mm(acc, half, lcol, rt, rsel, rcol, w):
                if DR:
                    for kp in range(KT // 2):
                        nc.tensor.matmul(
                            acc[:], hl_slice(half, kp, lcol, 128),
                            rt[:, rsel, 2 * kp:2 * kp + 2, rcol:rcol + w],
                            start=(kp == 0), stop=(kp == KT // 2 - 1),
                            perf_mode=mybir.MatmulPerfMode.DoubleRow)
                else:
                    for k in range(KT):
                        if with_pm:
                            lh = hl_t[:, half, k, lcol:lcol + 128]
                        else:
                            lh = hl_t[:, k, lcol:lcol + 128]
                        nc.tensor.matmul(
                            acc[:], lh, rt[:, rsel, k, rcol:rcol + w],
                            start=(k == 0), stop=(k == KT - 1))

            if with_pm:
                # scores: diag blocks of (own poses)^T @ g, quarter psums;
                # q0 extracts on DVE, q1 on DVE-mult + ACT accum-copies
                for hh in range(2):
                    for q in range(2):
                        acc_s = ps.tile([128, 512], mybir.dt.float32,
                                        tag="ps", name=f"accs{hh}{q}")
                        mm(acc_s[:, 0:256], 0, 128 * hh, g_t, hh,
                           256 * q, 256)
                        z = scr.tile([128, 256], mybir.dt.float32, tag="z",
                                     name=f"z{hh}{q}")
                        nc.vector.tensor_tensor(
                            z[:], acc_s[:, 0:256],
                            mask_t[:, 256 * q:256 * (q + 1)],
                            op=mybir.AluOpType.mult)
                        if q == 0:
                            nc.vector.tensor_reduce(
                                sc_t[:, 4 * hh:4 * hh + 2],
                                z[:].rearrange("p (m w) -> p m w", m=2),
                                axis=mybir.AxisListType.X,
                                op=mybir.AluOpType.add)
                        else:
                            zd = scr.tile([128, 128], mybir.dt.float32,
                                          tag="zd", name=f"zd{hh}")
                            for m in range(2):
                                nc.scalar.activation(
                                    zd[:], z[:, 128 * m:128 * (m + 1)],
                                    mybir.ActivationFunctionType.Copy,
                                    accum_out=sc_t[:, 4 * hh + 2 + m:
                                                   4 * hh + 3 + m])
                nc.sync.dma_start(sc_out, sc_t[:])

                # pm block: hl^T @ hr; evacs alternate ACT/DVE with a 1/256
                # descale, one small out-DMA per row-block
                for nb in range(2):
                    o = ev.tile([128, 4, 512], mybir.dt.float8e4, tag="ev",
                                name=f"ev{nb}")
                    for mb in range(4):
                        acc = ps.tile([128, 512], mybir.dt.float32, tag="ps",
                                      name=f"acc{nb}{mb}")
                        mm(acc, mb // 2, 128 * (mb % 2), hr_t, nb, 0, 512)
                        if mb % 2 == 0:
                            nc.scalar.activation(
                                o[:, mb], acc[:],
                                mybir.ActivationFunctionType.Copy,
                                scale=1.0 / (SC * SC))
                        else:
                            nc.vector.tensor_scalar_mul(
                                o[:, mb], acc[:], 1.0 / (SC * SC))
                        nc.sync.dma_start(
                            pmo[:, 512 * nb:512 * (nb + 1)]
                            .rearrange("(mb p) n -> p mb n", p=128)
                            [:, mb:mb + 1], o[:, mb:mb + 1])
            else:
                # packed hard-positives: masked dots, reduce-max per pose
                for hh in range(2):
                    acc_s = ps.tile([128, W2], mybir.dt.float32, tag="ps",
                                    name=f"accm{hh}")
                    mm(acc_s, 0, 128 * hh, g_t, hh, 0, W2)
                    z = scr.tile([128, W2], mybir.dt.float32, tag="z",
                                 name=f"zm{hh}")
                    nc.vector.tensor_tensor(
                        z[:], acc_s[:], mask_t[:, W2 * hh:W2 * (hh + 1)],
                        op=mybir.AluOpType.add)
                    nc.vector.tensor_reduce(
                        mx_t[:, hh:hh + 1], z[:],
                        axis=mybir.AxisListType.X, op=mybir.AluOpType.max)
                nc.sync.dma_start(mx_out, mx_t[:])

    nc.finalize()
    return nc


def _make_runner(nc):
    """Reusable jitted SPMD runner (replicates bass2jax.run_bass_via_pjrt but
    caches the compiled executable across calls)."""
    import jax
    from jax.sharding import Mesh, PartitionSpec
    from jax.experimental.shard_map import shard_map

    install_neuronx_cc_hook()
    partition_name = nc.partition_id_tensor.name if nc.partition_id_tensor else None
    in_names, out_names, out_avals = [], [], []
    for alloc in nc.m.functions[0].allocations:
        if not isinstance(alloc, mybir.MemoryLocationSet):
            continue
        name = alloc.memorylocations[0].name
        if alloc.kind == "ExternalInput":
            if name != partition_name:
                in_names.append(name)
        elif alloc.kind == "ExternalOutput":
            out_names.append(name)
            out_avals.append(jax.core.ShapedArray(
                tuple(alloc.tensor_shape), mybir.dt.np(alloc.dtype)))
    n_params = len(in_names)
    all_in = in_names + out_names + ([partition_name] if partition_name else [])

    def _body(*args):
        operands = list(args)
        if partition_name is not None:
            operands.append(partition_id_tensor())
        outs = _bass_exec_p.bind(
            *operands, out_avals=tuple(out_avals), in_names=tuple(all_in),
            out_names=tuple(out_names), lowering_input_output_aliases=(),
            sim_require_finite=False, sim_require_nnan=False, nc=nc)
        return tuple(outs)

    devices = jax.devices()[:NCORES]
    mesh = Mesh(np.asarray(devices), ("core",))
    donate = tuple(range(n_params, n_params + len(out_names)))
    sharded = jax.jit(
        shard_map(_body, mesh=mesh,
                  in_specs=(PartitionSpec("core"),) * (n_params + len(out_names)),
                  out_specs=(PartitionSpec("core"),) * len(out_names),
                  check_rep=False),
        donate_argnums=donate, keep_unused=True)

    def run(in_maps):
        concat_in = [np.concatenate([np.asarray(m[name]) for m in in_maps], axis=0)
                     for name in in_names]
        zeros = [np.zeros((NCORES * a.shape[0], *a.shape[1:]), a.dtype)
                 for a in out_avals]
        out_arrs = sharded(*concat_in, *zeros)
        return [
            {name: np.asarray(out_arrs[i]).reshape(NCORES, *out_avals[i].shape)[c]
             for i, name in enumerate(out_names)}
            for c in range(NCORES)
        ]

    return run


def _get_runner(key):
    if key not in _runners:
        _runners[key] = _make_runner(_build_kernel(with_pm=(key == "k1")))
    return _runners[key]


def _swz(x):
    """[CD, W] (uint8/fp8) -> [128, KT, W] with partition p holding
    contraction rows k*128+p."""
    return np.ascontiguousarray(x.reshape(KT, 128, x.shape[1]).transpose(1, 0, 2))


def _swz2(x):
    """[CD, 2*W] -> [128, 2, KT, W]: like _swz but with the two column
    halves split out as a leading chunk axis (contiguous DMA chunks)."""
    w = x.shape[1] // 2
    return np.ascontiguousarray(
        x.reshape(KT, 128, 2, w).transpose(1, 2, 0, 3))


def _kernel_host_fallback(inputs):
    """Pure-numpy reference replication, used only if the index tensors do
    not have the canonical arange structure the device layout relies on."""
    nlp = np.asarray(inputs["nlp_features"], np.float32)
    pose = np.asarray(inputs["pose_features"], np.float32)
    nlab = np.asarray(inputs["nlp_label"]).astype(np.int64)
    n2p = np.asarray(inputs["nlpid2poseid"]).astype(np.int64)
    p2n = np.asarray(inputs["pose2nlpid"]).astype(np.int64)
    cat = np.asarray(inputs["categories"], np.float32)
    ri = np.asarray(inputs["rand_index"]).astype(np.int64)
    Np, Nn = pose.shape[1], nlp.shape[1]
    norm_p = np.sqrt(np.einsum("cpd,cpd->cp", pose, pose, dtype=np.float32))
    norm_n = np.sqrt(np.einsum("cnd,cnd->cn", nlp, nlp, dtype=np.float32))
    poseF = pose / norm_p[:, :, None]
    nlpF = nlp / norm_n[:, :, None]
    loss_norm = np.float32(np.float32(norm_p.mean()) + np.float32(norm_n.mean()))
    dots = np.einsum("cnd,cnd->cn", nlpF, poseF[:, n2p]).astype(np.float32)
    scores = np.einsum("cn,nc->n", dots, cat).astype(np.float32)
    p = (1.0 / (1.0 + np.exp(-scores))).astype(np.float32)
    lblf = nlab.astype(np.float32)
    loss_label = np.float32(
        np.mean(-(np.log(p) * lblf + np.log(1.0 - p) * (1.0 - lblf))))
    pf = np.ascontiguousarray(poseF.transpose(0, 2, 1).reshape(-1, Np))
    pm = (pf.T @ pf).astype(np.float32)
    ar = np.arange(Np)
    pm[ar, ar] = 1.0
    order = np.argsort(pm, axis=1, kind="stable")
    furthest = order[ar, ri]
    sg = scores[p2n]
    lg = nlab[p2n]
    maxp = np.maximum(np.max(np.where(lg == 0, sg, -np.inf), axis=1), -1.0)
    minp = np.minimum(np.min(np.where(lg == 1, sg, np.inf), axis=1), 1.0)
    nids = p2n[furthest]
    cd = np.einsum("cpkd,cpd->cpk", nlpF[:, nids], poseF)
    cur = np.einsum("cpk,pkc->pk", cd, cat[nids]).astype(np.float32)
    lcur = nlab[nids]
    maxcur = np.max(np.where(lcur == 1, cur, -np.inf), axis=1)
    maxp = np.maximum(maxp, maxcur)
    found = ~((maxp == -1.0) | (minp == 1.0))
    lt = np.where(found, maxp - minp + 2.0, 0.0).astype(np.float32)
    nf = int(np.sum(~found))
    loss_triple = (np.float32(0.0) if nf == Nn else
                   np.float32(lt.sum(dtype=np.float32) / np.float32(Nn - nf)))
    return (np.float32(loss_label), loss_norm, loss_triple)


def kernel(**inputs):
    nlp = np.ascontiguousarray(inputs["nlp_features"], np.float32)      # [C, NN, D]
    pose = np.ascontiguousarray(inputs["pose_features"], np.float32)    # [C, NP, D]
    nlab = np.asarray(inputs["nlp_label"]).astype(np.int64)
    cat = np.ascontiguousarray(inputs["categories"], np.float32)        # [NN, C]
    ri = np.asarray(inputs["rand_index"]).astype(np.int64)

    n2p = np.asarray(inputs["nlpid2poseid"]).astype(np.int64)
    p2n = np.asarray(inputs["pose2nlpid"]).astype(np.int64)
    if (not np.array_equal(n2p, np.arange(NN) // K)
            or not np.array_equal(p2n, np.arange(NN).reshape(NP, K))):
        return _kernel_host_fallback(inputs)

    # ---- host: exact norms, normalize, fold categories, fp8 x16 ---------
    norm_p = np.sqrt(np.einsum("cpd,cpd->cp", pose, pose, dtype=np.float32,
                               optimize=True)).astype(np.float32)       # [C, NP]
    norm_n = np.sqrt(np.einsum("cnd,cnd->cn", nlp, nlp, dtype=np.float32,
                               optimize=True)).astype(np.float32)       # [C, NN]
    loss_norm = np.float32(np.float32(norm_p.mean()) + np.float32(norm_n.mean()))

    poseF = pose / norm_p[:, :, None]
    hT8 = np.ascontiguousarray(
        (poseF * SC).transpose(0, 2, 1)).reshape(CD, NP).astype(F8)     # [CD, NP]

    gscale = (cat.T / norm_n) * SC                                      # [C, NN]
    g8 = (nlp * gscale[:, :, None]).astype(F8)                          # [C, NN, D]
    g8T = np.ascontiguousarray(g8.transpose(0, 2, 1)).reshape(CD, NN)   # [CD, NN]
    # (p, m) -> (m, p) within each 512-column block so the device's diag
    # mask lines up: col 512*hh + 128*m + p <- local row 512*hh + 4*p + m
    g8km = np.ascontiguousarray(
        g8T.reshape(CD, NN // 512, 128, 4).transpose(0, 1, 3, 2)
    ).reshape(CD, NN)

    mask = np.zeros((128, 512), np.float32)
    mask[np.arange(128)[:, None], 128 * np.arange(4)[None, :] + np.arange(128)[:, None]] = 1.0
    mask = mask.astype(ml_dtypes.bfloat16)

    # ---- device kernel 1 -------------------------------------------------
    # hl column order per core: own 256 poses first (so the scores matmul
    # can address them at a fixed offset), then the other half of the pm
    # row-block; pm rows are written back through rows_order.
    run1 = _get_runner("k1")
    in1 = []
    rows_order = np.empty((NCORES, PM_M), np.int64)
    for c in range(NCORES):
        i, j = c // PM_NJ, c % PM_NJ
        par = c % 2
        own = np.arange(512 * i + 256 * par, 512 * i + 256 * par + 256)
        oth = np.arange(512 * i + 256 * (1 - par), 512 * i + 256 * (1 - par) + 256)
        cols = np.concatenate([own, oth])
        rows_order[c] = cols
        in1.append({
            "g": _swz2(g8km[:, c * NNL:(c + 1) * NNL]),
            "hl": _swz2(hT8[:, cols]),
            "hr": _swz2(hT8[:, PM_N * j:PM_N * (j + 1)]),
            "mask": mask,
        })
    res1 = run1(in1)

    # ---- host: scores / BCE ---------------------------------------------
    sc_all = np.stack([r["sc"] for r in res1])                          # [8,128,8]
    scores = (sc_all.reshape(NCORES, 128, 2, 4).transpose(0, 2, 1, 3)
              .reshape(NN) / np.float32(SC * SC)).astype(np.float32)
    p = (1.0 / (1.0 + np.exp(-scores))).astype(np.float32)
    lblf = nlab.astype(np.float32)
    loss_label = np.float32(
        np.mean(-(np.log(p) * lblf + np.log(1.0 - p) * (1.0 - lblf))))

    # ---- host: furthest selection ---------------------------------------
    pm = np.empty((NP, NP), np.float32)
    for c in range(NCORES):
        j = c % PM_NJ
        pm[rows_order[c], PM_N * j:PM_N * (j + 1)] = \
            res1[c]["pm8"].astype(np.float32)
    ar = np.arange(NP)
    pm[ar, ar] = np.float32(1.0)
    order = np.argsort(pm, axis=1, kind="stable")
    furthest = order[ar, ri]                                            # [NP]

    sg = scores.reshape(NP, K)
    lg = nlab.reshape(NP, K)
    maxp = np.maximum(np.max(np.where(lg == 0, sg, -np.inf), axis=1), -1.0)
    minp = np.minimum(np.min(np.where(lg == 1, sg, np.inf), axis=1), 1.0)

    # ---- device kernel 2: packed label-1 hard-positive columns ----------
    f4 = furthest[:, None] * K + np.arange(K)                           # [NP, K]
    lab4 = nlab[f4] == 1                                                # [NP, K]
    mflat = lab4.reshape(NCORES, 2, 512)                                # (c,hh,(p,k))
    pos = np.cumsum(mflat, axis=2) - 1
    valid = mflat & (pos < W2)
    ci, hi, ei = np.nonzero(valid)
    pcols = pos[valid]
    src = f4.reshape(NCORES, 2, 512)[valid]                             # g rows
    g2u = np.zeros((CD, NCORES, 2, W2), np.uint8)
    g2u[:, ci, hi, pcols] = g8T.view(np.uint8)[:, src]
    maskn = np.full((NCORES, 128, 2, W2), NEG, np.float32)
    maskn[ci, ei // K, hi, pcols] = 0.0
    maskn = maskn.reshape(NCORES, 128, 2 * W2).astype(ml_dtypes.bfloat16)

    run2 = _get_runner("k2")
    in2 = []
    for c in range(NCORES):
        gc = g2u[:, c].reshape(CD, 2 * W2)
        in2.append({
            "g": _swz2(gc).view(F8),
            "hl": _swz(hT8[:, NPL * c:NPL * (c + 1)]),
            "maskn": maskn[c],
        })
    res2 = run2(in2)
    mx = np.stack([r["mx"] for r in res2])                              # [8,128,2]
    maxcur = np.where(mx > -1.0e8, mx / np.float32(SC * SC), -np.inf)
    maxcur = maxcur.transpose(0, 2, 1).reshape(NP)                      # (c,hh,p)

    # overflowed packed columns (> W2 label-1 entries per half): host dots
    if valid.sum() != lab4.sum():
        off = mflat & (pos >= W2)
        for c0, h0, e0 in zip(*np.nonzero(off)):
            q = c0 * NPL + h0 * 128 + e0 // K
            r = f4.reshape(NCORES, 2, 512)[c0, h0, e0]
            v = float(np.dot(g8T[:, r].astype(np.float32),
                             hT8[:, q].astype(np.float32))) / (SC * SC)
            maxcur[q] = max(maxcur[q], v)

    maxp = np.maximum(maxp, maxcur)
    found = ~((maxp == -1.0) | (minp == 1.0))
    lt = np.where(found, maxp - minp + 2.0, 0.0).astype(np.float32)
    not_find = int(np.sum(~found))
    if not_find == NN:
        loss_triple = np.float32(0.0)
    else:
        loss_triple = np.float32(lt.sum(dtype=np.float32) / np.float32(NN - not_find))

    return (np.float32(loss_label), np.float32(loss_norm), np.float32(loss_triple))
